# revision 47
# baseline (speedup 1.0000x reference)
"""Trainium2 Bass kernel for a dense transformer block (B=2, T=2048, C=1024,
H=16, DFF=4096), distributed over 8 NeuronCores.

Sharding: 2 batch groups x 4-way query-block sharding. Core c handles batch
g=c//4 and query blocks {j, 7-j} (j=c%4). K/V are computed per-core for the
full batch (replicated; no collectives), with the batch rows PERMUTED per
core so the core's own query blocks come first: xc = [block j, block 7-j,
remaining blocks ascending]. Softmax/AV are permutation-invariant over keys,
so only the (per-core input) masks need to know the order. This makes the
first 512 rows simultaneously the query rows: LN1 runs once over 2048 rows
(no duplicated query pass), Q projects from x1T[0], and the attention
residual reads x1T[0] directly.

LayerNorm gammas/betas are folded host-side into the consuming weights
(W' = gamma*W row-scaled, b' = b + beta@W), so the device only produces the
plain normalized x-hat; the two residual adds apply gamma/beta feature-major
where they are per-partition scalars. Transposes run in bf16 (1 cyc/row),
batched 8-to-a-psum-bank with a single copy out. Scores are computed in
2-bank rounds with one exp per round; softmax denominators ride the AV
matmul as a ones-column of V; the reciprocal is broadcast on GPSIMD.

Causality at 256-block granularity: with the permuted key order, chunk pairs
{0,1},{4..9} feed both query halves (one N=512 matmul), {2,3},{10..15} feed
only the late half. Per-core 0/1/triangular masks (inputs) make one NEFF
serve all 8 cores (SPMD).
"""
import numpy as np
import ml_dtypes

import concourse.bass as bass
import concourse.mybir as mybir
import concourse.tile as tile
from concourse.vector_clock import ScopedClock
from concourse.bass_utils import run_bass_kernel_spmd
from concourse.masks import make_identity

bf16 = ml_dtypes.bfloat16
f32 = mybir.dt.float32
bt16 = mybir.dt.bfloat16
AF = mybir.ActivationFunctionType
OP = mybir.AluOpType

B, T, C, H, DH, DFF = 2, 2048, 1024, 16, 64, 4096
P = 128
QB = 256            # rows per query block
R = 512             # own query rows per core
CC = C // P         # 8 feature chunks
MM = DFF // P       # 32 ffn chunks
EPS = 1e-5

# score chunk index sets (permuted key order, see module docstring)
BOTH = (0, 1, 4, 5, 6, 7, 8, 9)           # N=512: both query halves
LATE = (2, 3, 10, 11, 12, 13, 14, 15)     # N=256: late half only
BOTH_R = ((0, 1), (4, 5), (6, 7), (8, 9))
LATE_R = ((2, 3, 10, 11), (12, 13, 14, 15))


# ---------------------------------------------------------------------------
# The walrus build in this container rejects instructions with >1 sync wait.
# Tile's sem assignment can emit several on one instruction; split the excess
# onto same-engine NoOps placed immediately before.
def _patched_drain_and_barrier(self, tick_clock, wait_clock):
    nc = self.nc
    probe = nc.sync.nop(nofuse=True, hint="tail_wait_probe")
    wait_clock.add_sem_waits(probe.ins, ScopedClock({None: tick_clock.global_clock}))
    si = probe.ins.sync_info
    waits = list(si.on_wait) if si is not None else []
    if si is not None:
        si.on_wait = waits[:1]
    for w in waits[1:]:
        n2 = nc.sync.nop(nofuse=True, hint="tail_wait_split")
        n2.ins.sync_info = mybir.SyncInfo(on_wait=[w], on_update=[])
    nc.sync.drain()
    nc.all_engine_barrier()
    assert self.sems is not None
    popped = nc._tile_sem_poison_stack.pop()
    assert popped is self._sem_poison
    nc.clear_and_free_semaphores(list(self.sems.allocated().values()))
    nc.all_engine_barrier()


tile.TileContext._drain_and_barrier = _patched_drain_and_barrier

_MAX_WAITS = 1
_split_counter = [0]


def _split_sync_waits(nc):
    for fn in nc.m.functions:
        for bb in fn.blocks:
            new_insts = []
            for inst in bb.instructions:
                si = getattr(inst, "sync_info", None)
                lim = _MAX_WAITS
                if si is not None and si.on_wait and len(si.on_wait) > lim:
                    waits = list(si.on_wait)
                    keep = waits[-lim:]
                    excess = waits[:-lim]
                    for i in range(0, len(excess), _MAX_WAITS):
                        _split_counter[0] += 1
                        nop = mybir.InstNoOp(
                            name=f"I-wsplit-{_split_counter[0]}", ins=[], outs=[])
                        nop.engine = inst.engine
                        nop.sync_info = mybir.SyncInfo(
                            on_wait=excess[i:i + _MAX_WAITS], on_update=[])
                        new_insts.append(nop)
                    si.on_wait = keep
                new_insts.append(inst)
            bb.instructions = new_insts
# ---------------------------------------------------------------------------


class Ctx:
    pass


def _ln_stats(g, xt, n_feat):
    """bn stats for one row-major tile; returns (mean[P,1], rstd[P,1])."""
    nc = g.nc
    st = g.stats.tile([P, 2, 6], f32, tag="bnst", name="bnst")
    xv = xt.rearrange("p (s d) -> p s d", s=2)
    for sg in range(2):
        nc.vector.bn_stats(out=st[:, sg, :], in_=xv[:, sg, :])
    mv = g.stats.tile([P, 2], f32, tag="bnmv", name="bnmv")
    nc.vector.bn_aggr(out=mv[:], in_=st[:])
    sq = g.stats.tile([P, 1], f32, tag="bnsq", name="bnsq")
    nc.scalar.activation(out=sq[:], in_=mv[:, 1:2], func=AF.Sqrt,
                         bias=g.eps_sb[:], scale=float(n_feat) / (n_feat - 1))
    rstd = g.stats.tile([P, 1], f32, tag="bnrstd", name="bnrstd")
    nc.vector.reciprocal(rstd[:], sq[:])
    return mv, rstd


def _ln_tile_to_fm(g, xt, out_writes, tag="xn"):
    """LN a row-major [P, C] tile to x-hat, transpose to feature-major via 8
    bf16 transposes into one psum bank, and hand the [P, CC, P] psum view to
    out_writes for the single batched copy out."""
    nc = g.nc
    mv, rstd = _ln_stats(g, xt, C)
    xn = g.xnp.tile([P, C], bt16, tag=tag, name=tag)
    nc.vector.tensor_scalar(out=xn[:], in0=xt[:], scalar1=mv[:, 0:1],
                            scalar2=rstd[:], op0=OP.subtract, op1=OP.mult)
    pt = g.ps.tile([P, C], bt16, tag="pav", name="pt_t")
    for c in range(CC):
        nc.tensor.transpose(pt[:, c * P:(c + 1) * P], xn[:, c * P:(c + 1) * P],
                            g.ident[:])
    out_writes(pt.rearrange("p (c q) -> p c q", c=CC))


def _phase_a(g):
    """LN1 over the (permuted) batch + V per row-block."""
    nc = g.nc
    for kt in range(T // P):
        nc.vector.memset(g.vv[kt][:, :, DH:DH + 1], 1.0)
    for rt in range(T // P):
        rb, r0 = rt // 4, (rt % 4) * P
        if rt < len(g.xt_pre):
            xt = g.xt_pre[rt]
        else:
            xt = g.xio.tile([P, C], f32, tag="xin", name="xin")
            nc.sync.dma_start(xt[:], g.xc[rt * P:(rt + 1) * P, :])

        def wr1(pv, rb=rb, r0=r0):
            nc.scalar.copy(out=g.x1T[rb][:, :, r0:r0 + P], in_=pv)
        _ln_tile_to_fm(g, xt, wr1)

        # V for this tile's keys (kt == rt): needs only these 128 columns
        kt, k0 = rt, r0
        for half in range(2):
            pv = g.ps.tile([P, 512], f32, tag="pp", name="ps_v")
            for c in range(CC):
                nc.tensor.matmul(
                    pv[:], g.x1T[rb][:, c, k0:k0 + P],
                    g.wvs[:, c, half * 512:(half + 1) * 512],
                    start=(c == 0), stop=(c == CC - 1))
            nc.vector.tensor_copy(
                out=g.vv[kt][:, half * 8:(half + 1) * 8, 0:DH],
                in_=pv.rearrange("p (h d) -> p h d", h=8))


def _kq_chains(g, m):
    """Issue wk/wq DMAs for pair m; return (kT, qT, [chain closures]).
    Each closure issues one 8-matmul projection chain (PE filler work)."""
    nc = g.nc
    wkm = g.wstr.tile([P, CC, P], bt16, tag="wkm", name="wkm")
    nc.sync.dma_start(wkm[:], g.wk[:, m])
    wqm = g.wstr.tile([P, CC, P], bt16, tag="wqm", name="wqm")
    nc.sync.dma_start(wqm[:], g.wq[:, m])
    kT = g.kvq.tile([P, 4, 512], bt16, tag="kT", name="kT")
    qT = g.kvq.tile([P, 512], bt16, tag="qT", name="qT")

    def k_chain(rb):
        pk = g.ps.tile([P, 512], f32, tag="pp", name="ps_k")
        for c in range(CC):
            nc.tensor.matmul(pk[:], wkm[:, c, :], g.x1T[rb][:, c, :],
                             start=(c == 0), stop=(c == CC - 1))
        nc.vector.tensor_scalar(
            out=kT[:, rb, :], in0=pk[:],
            scalar1=g.vec["bk"][:, m:m + 1], scalar2=None, op0=OP.add)

    def q_chain():
        pq = g.ps.tile([P, 512], f32, tag="pp", name="ps_q")
        for c in range(CC):
            nc.tensor.matmul(pq[:], wqm[:, c, :], g.x1T[0][:, c, :],
                             start=(c == 0), stop=(c == CC - 1))
        nc.scalar.activation(out=qT[:], in_=pq[:], func=AF.Identity,
                             bias=g.vec["bq"][:, m:m + 1], scale=1.0)

    chains = [lambda rb=rb: k_chain(rb) for rb in range(4)] + [q_chain]
    return kT, qT, chains


def _attn_head(g, kT, qT, m, hl, filler, deferred_norm):
    """Scores + exp + mask + AV for head h=2m+hl, software-pipelined at round
    granularity: AV of round r-1 issues after scores of round r, `filler`
    (next pair's K/Q chains) plugs PE stalls, and the previous head's softmax
    normalization (latency-bound) runs mid-head via `deferred_norm`."""
    nc = g.nc
    h = 2 * m + hl
    hs = slice(hl * DH, (hl + 1) * DH)
    rounds = []
    pav = g.ps.tile([P, 512], f32, tag="pav", name="ps_av")

    def score_round(r):
        psc = g.ps.tile([P, 2, 512], f32, tag="psc", name="ps_s")
        if r < 4:
            for i, kc in enumerate(BOTH_R[r]):
                rb, k0 = kc // 4, (kc % 4) * P
                nc.tensor.matmul(psc[:, i, :], kT[hs, rb, k0:k0 + P],
                                 qT[hs, :], start=True, stop=True,
                                 tile_position=(hl * DH, 0))
            aA = g.arnd.tile([P, 2, 512], bt16, tag="arnd", name="aA")
            nc.scalar.activation(out=aA[:], in_=psc[:], func=AF.Exp)
            nc.vector.tensor_mul(aA[:, :, 0:QB], aA[:, :, 0:QB],
                                 g.mq[:, 2 * r:2 * r + 2, :])
        else:
            for i, kc in enumerate(LATE_R[r - 4]):
                rb, k0 = kc // 4, (kc % 4) * P
                nc.tensor.matmul(
                    psc[:, i // 2, (i % 2) * QB:(i % 2 + 1) * QB],
                    kT[hs, rb, k0:k0 + P], qT[hs, QB:512],
                    start=True, stop=True, tile_position=(hl * DH, 0))
            aA = g.arnd.tile([P, 2, 512], bt16, tag="arnd", name="aB")
            nc.scalar.activation(out=aA[:], in_=psc[:], func=AF.Exp)
            ab4 = aA.rearrange("p a (b q) -> p (a b) q", b=2)
            nc.vector.tensor_mul(ab4[:], ab4[:],
                                 g.mq[:, 4 * r - 8:4 * r - 4, :])
        rounds.append(aA)

    def av_round(r):
        if r < 4:
            for i, kc in enumerate(BOTH_R[r]):
                nc.tensor.matmul(pav[:DH + 1, :], g.vv[kc][:, h, :],
                                 rounds[r][:, i, :], start=(r == 0 and i == 0),
                                 stop=False)
        else:
            for i, kc in enumerate(LATE_R[r - 4]):
                last = (r == 5) and (i == 3)
                nc.tensor.matmul(
                    pav[:DH + 1, QB:512], g.vv[kc][:, h, :],
                    rounds[r][:, i // 2, (i % 2) * QB:(i % 2 + 1) * QB],
                    start=False, stop=last)

    for r in range(6):
        score_round(r)
        if r == 1:
            deferred_norm()       # previous head's normalization
        if r >= 1:
            av_round(r - 1)
            filler()
    av_round(5)
    filler()

    def norm():
        rr = g.stats.tile([1, 512], bt16, tag="rr", name="rr")
        with nc.allow_low_precision(reason="1/den in bf16 is enough"):
            nc.vector.reciprocal(rr[:], pav[DH:DH + 1, :])
        pr = g.ps.tile([P, 2, 512], f32, tag="psc", name="ps_r")
        nc.tensor.matmul(pr[:DH, 0, :], g.ones64[:], rr[:],
                         start=True, stop=True)
        rbc = g.rbcp.tile([DH, 512], bt16, tag="rbc", name="rbc")
        nc.vector.tensor_copy(out=rbc[:], in_=pr[:DH, 0, :])
        nc.vector.tensor_mul(out=g.hcat[m][hs, :], in0=pav[:DH, :],
                             in1=rbc[:])
    return norm


def _phase_c(g):
    """Wo + residual -> x2T (bf16), with LN2 stats accumulated feature-major
    on the fly (column sums of x2 and x2^2 via ones-vector matmuls), then
    x3h = (x2 - mu) * rstd via broadcasted [1,512] stats. No transposes."""
    nc = g.nc
    wtiles = []
    for mo in range(3):
        wos = g.wstr.tile([P, CC, P], bt16, tag="wos", name="wos", bufs=3)
        nc.sync.dma_start(wos[:], g.wo[:, mo])
        wtiles.append(wos)
    pstat = g.ps.tile([P, 2, 512], f32, tag="psc", name="pstat")
    for mo in range(CC):
        wos = wtiles[mo]
        if mo + 3 < CC:
            nw = g.wstr.tile([P, CC, P], bt16, tag="wos", name="wos", bufs=3)
            nc.sync.dma_start(nw[:], g.wo[:, mo + 3])
            wtiles.append(nw)
        pa = g.ps.tile([P, 512], f32, tag="pp", name="ps_o")
        for c in range(CC):
            nc.tensor.matmul(pa[:], wos[:, c, :], g.hcat[c][:, :],
                             start=(c == 0), stop=(c == CC - 1))
        # x2 = gamma1 * xhat1 + (attn + bo + beta1)
        t = g.x2p.tile([P, 512], f32, tag="x2t", name="x2t")
        nc.vector.scalar_tensor_tensor(
            out=t[:], in0=g.x1T[0][:, mo, :], scalar=g.vec["g1"][:, mo:mo + 1],
            in1=pa[:], op0=OP.mult, op1=OP.add)
        nc.vector.tensor_scalar(
            out=g.x2T[mo][:], in0=t[:], scalar1=g.vec["bos"][:, mo:mo + 1],
            scalar2=None, op0=OP.add)
        sq = g.x2p.tile([P, 512], bt16, tag="sqt", name="sqt")
        nc.scalar.square(out=sq[:], in_=g.x2T[mo][:])
        nc.tensor.matmul(pstat[0:1, 0, :], g.ones128[:], g.x2T[mo][:],
                         start=(mo == 0), stop=(mo == CC - 1))
        nc.tensor.matmul(pstat[0:1, 1, :], g.ones128[:], sq[:],
                         start=(mo == 0), stop=(mo == CC - 1))
    # mu = S1/C ; var*(C-1) = S2 - C*mu^2 ; rstd = rsqrt(var + eps)
    mu = g.stats.tile([1, 512], f32, tag="mu", name="mu")
    nc.vector.tensor_scalar_mul(mu[:], pstat[0:1, 0, :], 1.0 / C)
    bsrc = g.stats.tile([1, 2, 512], f32, tag="bsrc", name="bsrc")
    musq = g.stats.tile([1, 512], f32, tag="musq", name="musq")
    nc.vector.tensor_mul(musq[:], mu[:], mu[:])
    var = g.stats.tile([1, 512], f32, tag="var", name="var")
    nc.vector.scalar_tensor_tensor(
        out=var[:], in0=musq[:], scalar=-float(C), in1=pstat[0:1, 1, :],
        op0=OP.mult, op1=OP.add)
    srt = g.stats.tile([1, 512], f32, tag="srt", name="srt")
    nc.scalar.activation(out=srt[:], in_=var[:], func=AF.Sqrt,
                         bias=g.eps_sb[0:1, :], scale=1.0 / (C - 1))
    nc.vector.reciprocal(bsrc[:, 0, :], srt[:])
    nc.vector.tensor_mul(bsrc[:, 1, :], mu[:], bsrc[:, 0, :])
    # broadcast rstd and mu*rstd to all partitions
    pb = g.ps.tile([P, 2, 512], f32, tag="psc", name="pb")
    nc.tensor.matmul(pb[:, 0, :], g.onesPf[:], bsrc[:, 0, :],
                     start=True, stop=True)
    nc.tensor.matmul(pb[:, 1, :], g.onesPf[:], bsrc[:, 1, :],
                     start=True, stop=True)
    nmsb = g.x2p.tile([P, 512], f32, tag="nmsb", name="nmsb")
    nc.scalar.copy(out=nmsb[:], in_=pb[:, 1, :])
    for c in range(CC):
        t1 = g.x2p.tile([P, 512], f32, tag="x2t", name="x3t")
        nc.vector.tensor_mul(t1[:], g.x2T[c][:], pb[:, 0, :])
        # subtract on GPSIMD so W1's first chain is not DVE-paced
        nc.gpsimd.tensor_sub(g.x3h[:, c, :], t1[:], nmsb[:])


def _phase_d(g):
    """LN2 (to x-hat, gamma2/beta2 folded into W1/outputs) + FFN + out."""
    nc, tc = g.nc, g.tc
    with tc.tile_pool(name="dp", bufs=1) as dp, \
         tc.tile_pool(name="w1p", bufs=4) as w1p, \
         tc.tile_pool(name="outp", bufs=2) as outp, \
         tc.tile_pool(name="w2s", bufs=2) as w2s:
        g.outp, g.w2s = outp, w2s
        _phase_d_body(g, dp, w1p)


def _phase_d_body(g, dp, w1p):
    nc = g.nc
    w1tiles = []
    for mm in range(3):
        w1m = w1p.tile([P, CC, P], bt16, tag="w1m", name="w1m")
        nc.sync.dma_start(w1m[:], g.w1[:, mm])
        w1tiles.append(w1m)

    h1 = [dp.tile([P, 512], bt16, tag=f"h1_{mm}", name=f"h1_{mm}")
          for mm in range(MM)]
    for mm in range(MM):
        w1m = w1tiles[mm]
        if mm + 3 < MM:
            nw = w1p.tile([P, CC, P], bt16, tag="w1m", name="w1m")
            nc.sync.dma_start(nw[:], g.w1[:, mm + 3])
            w1tiles.append(nw)
        if mm == MM - 2:
            # prefetch the first W2 chunks behind the last W1 loads
            w2tiles = []
            for oc in range(2):
                w2m = g.w2s.tile([P, MM, P], bt16, tag="w2m", name="w2m")
                nc.sync.dma_start(w2m[:], g.w2[:, oc])
                w2tiles.append(w2m)
        p1 = g.ps.tile([P, 512], f32, tag="pp", name="ps_f1")
        for c in range(CC):
            nc.tensor.matmul(p1[:], w1m[:, c, :], g.x3h[:, c, :],
                             start=(c == 0), stop=(c == CC - 1))
        nc.scalar.activation(out=h1[mm][:], in_=p1[:], func=AF.Gelu,
                             bias=g.vec["b1"][:, mm:mm + 1], scale=1.0)
    for oc in range(CC):
        w2m = w2tiles[oc]
        if oc + 2 < CC:
            nw = g.w2s.tile([P, MM, P], bt16, tag="w2m", name="w2m")
            nc.sync.dma_start(nw[:], g.w2[:, oc + 2])
            w2tiles.append(nw)
        p2 = g.ps.tile([P, 512], f32, tag="pp", name="ps_f2")
        for k in range(MM):
            nc.tensor.matmul(p2[:], w2m[:, k, :], h1[k][:, :],
                             start=(k == 0), stop=(k == MM - 1))
        # out = gamma2 * xhat2 + (ffw + b2 + beta2)
        t = g.x2p.tile([P, 512], f32, tag="x2t", name="x2o")
        nc.vector.scalar_tensor_tensor(
            out=t[:], in0=g.x3h[:, oc, :], scalar=g.vec["g2"][:, oc:oc + 1],
            in1=p2[:], op0=OP.mult, op1=OP.add)
        ot = g.outp.tile([P, 512], f32, tag="otile", name="otile")
        nc.vector.tensor_scalar(
            out=ot[:], in0=t[:], scalar1=g.vec["b2s"][:, oc:oc + 1],
            scalar2=None, op0=OP.add)
        nc.sync.dma_start(g.out[oc], ot[:])


def build_kernel():
    nc = bass.Bass("TRN2", target_bir_lowering=False, num_devices=8)
    g = Ctx()
    g.nc = nc

    g.xc = nc.dram_tensor("xc", [T, C], f32, kind="ExternalInput").ap()
    g.wk = nc.dram_tensor("wk", [P, CC, CC, P], bt16, kind="ExternalInput").ap()
    g.wq = nc.dram_tensor("wq", [P, CC, CC, P], bt16, kind="ExternalInput").ap()
    g.wv = nc.dram_tensor("wv", [P, CC, C], bt16, kind="ExternalInput").ap()
    g.wo = nc.dram_tensor("wo", [P, CC, CC, P], bt16, kind="ExternalInput").ap()
    g.w1 = nc.dram_tensor("w1", [P, MM, CC, P], bt16, kind="ExternalInput").ap()
    g.w2 = nc.dram_tensor("w2", [P, CC, MM, P], bt16, kind="ExternalInput").ap()
    g.masks = nc.dram_tensor("masks", [P, 16, QB], bt16,
                             kind="ExternalInput").ap()
    vecs = {}
    for nm, n in [("bq", CC), ("bk", CC), ("bos", CC), ("b1", MM),
                  ("b2s", CC), ("g1", CC), ("g2", CC)]:
        vecs[nm] = nc.dram_tensor(nm, [n, P], f32, kind="ExternalInput").ap()
    g.out = nc.dram_tensor("out", [CC, P, R], f32, kind="ExternalOutput").ap()

    with tile.TileContext(nc) as tc:
        g.tc = tc
        _build_body(g, vecs)
    _split_sync_waits(nc)
    return nc


def _build_body(g, vecs):
    nc, tc = g.nc, g.tc
    from contextlib import ExitStack
    with ExitStack() as es:
        pool = lambda name, bufs, **kw: es.enter_context(
            tc.tile_pool(name=name, bufs=bufs, **kw))
        setup = pool("setup", 1)
        g.setup = setup
        g.stats = pool("stats", 4)
        g.ps = pool("ps", 2, space="PSUM")
        g.xnp = pool("xnp", 2)
        g.rbcp = pool("rbcp", 2)
        hp = pool("hp", 1)
        g.x2p = pool("x2p", 2)
        x2Tp = pool("x2Tp", 1)
        x3p = pool("x3p", 1)

        ident = setup.tile([P, P], bt16, tag="ident", name="ident")
        make_identity(nc, ident[:])
        g.ident = ident
        g.eps_sb = setup.tile([P, 1], f32, tag="eps", name="eps")
        nc.vector.memset(g.eps_sb[:], EPS)
        g.ones64 = setup.tile([1, DH], bt16, tag="ones64", name="ones64")
        nc.vector.memset(g.ones64[:], 1.0)
        g.ones128 = setup.tile([P, 1], bt16, tag="ones128", name="ones128")
        nc.vector.memset(g.ones128[:], 1.0)
        g.onesPf = setup.tile([1, P], f32, tag="onesPf", name="onesPf")
        nc.vector.memset(g.onesPf[:], 1.0)
        # warm the PE p-state while the first x DMA is in flight
        pwarm = g.ps.tile([P, 2048], bt16, tag="psc", name="pwarm")
        for i in range(16):
            nc.tensor.transpose(pwarm[:, i * P:(i + 1) * P], ident[:],
                                ident[:])

        g.hcat = [hp.tile([P, 512], bt16, tag=f"hcat{c}", name=f"hcat{c}")
                  for c in range(CC)]
        g.x2T = [x2Tp.tile([P, 512], bt16, tag=f"x2T{mo}", name=f"x2T{mo}")
                 for mo in range(CC)]
        g.x3h = x3p.tile([P, CC, 512], bt16, tag="x3h", name="x3h")

        with ExitStack() as es2:
            pool2 = lambda name, bufs: es2.enter_context(
                tc.tile_pool(name=name, bufs=bufs))
            x1p = pool2("x1p", 1)
            kvp = pool2("kvp", 1)
            g.wstr = pool2("wstr", 2)
            g.x1T = [x1p.tile([P, CC, 512], bt16, tag=f"x1T{rb}",
                              name=f"x1T{rb}") for rb in range(4)]
            g.vv = [kvp.tile([P, H, DH + 1], bt16, tag=f"vv{kt}",
                             name=f"vv{kt}") for kt in range(T // P)]
            with ExitStack() as esa:
                poola = lambda name, bufs: esa.enter_context(
                    tc.tile_pool(name=name, bufs=bufs))
                g.xio = poola("xio", 3)
                wvp = poola("wvp", 1)
                # first xin DMAs go out before anything else
                g.xt_pre = []
                for rt in range(3):
                    xt = g.xio.tile([P, C], f32, tag="xin", name="xin")
                    nc.sync.dma_start(xt[:], g.xc[rt * P:(rt + 1) * P, :])
                    g.xt_pre.append(xt)
                g.wvs = wvp.tile([P, CC, C], bt16, tag="wvs", name="wvs")
                nc.sync.dma_start(g.wvs[:], g.wv)
                g.vec = {}
                for nm, ap_ in vecs.items():
                    n = ap_.shape[0]
                    t = g.setup.tile([P, n], f32, tag=f"vec_{nm}",
                                     name=f"vec_{nm}")
                    nc.sync.dma_start(t[:], ap_.rearrange("c p -> p c"))
                    g.vec[nm] = t
                _phase_a(g)
            with ExitStack() as esb:
                poolb = lambda name, bufs: esb.enter_context(
                    tc.tile_pool(name=name, bufs=bufs))
                g.kvq = poolb("kvq", 3)
                mp = poolb("mp", 1)
                g.arnd = poolb("arp", 6)
                g.mq = mp.tile([P, 16, QB], bt16, tag="mask", name="mask")
                nc.sync.dma_start(g.mq[:], g.masks)
                _phase_b(g)
                _phase_c(g)
        _phase_d(g)


def _phase_b(g):
    kq = {}
    pending = []          # [(pair_index, chain_closure), ...] in order

    def ensure(mi):
        if mi < CC and mi not in kq:
            kT, qT, chains = _kq_chains(g, mi)
            kq[mi] = (kT, qT)
            pending.extend((mi, ch) for ch in chains)

    def filler():
        if pending:
            pending.pop(0)[1]()

    norms = []

    def deferred_norm():
        if norms:
            norms.pop(0)()

    ensure(0)
    while pending:
        pending.pop(0)[1]()
    ensure(1)
    for m in range(CC):
        ensure(m + 2)
        kT, qT = kq[m]
        for hl in range(2):
            norms.append(_attn_head(g, kT, qT, m, hl, filler, deferred_norm))
        # everything pair m+1 references must be issued before its scores
        while pending and pending[0][0] <= m + 1:
            pending.pop(0)[1]()
    while norms:
        norms.pop(0)()


_NC_CACHE = None


def _get_nc():
    global _NC_CACHE
    if _NC_CACHE is None:
        _NC_CACHE = build_kernel()
    return _NC_CACHE


def _prep_shared(inputs):
    scale = DH ** -0.5
    Wq = np.asarray(inputs["Wq"], np.float32)
    Wk = np.asarray(inputs["Wk"], np.float32)
    Wv = np.asarray(inputs["Wv"], np.float32)
    Wo = np.asarray(inputs["Wo"], np.float32)
    W1 = np.asarray(inputs["W1"], np.float32)
    W2 = np.asarray(inputs["W2"], np.float32)
    g1 = np.asarray(inputs["gamma1"], np.float32)
    be1 = np.asarray(inputs["beta1"], np.float32)
    g2 = np.asarray(inputs["gamma2"], np.float32)
    be2 = np.asarray(inputs["beta2"], np.float32)

    Wk2 = np.ascontiguousarray(Wk.transpose(1, 0, 2).reshape(C, C))
    Wq2 = np.ascontiguousarray(Wq.transpose(1, 0, 2).reshape(C, C)) * scale
    Wv2 = np.ascontiguousarray(Wv.transpose(1, 0, 2).reshape(C, C))

    bk_eff = np.asarray(inputs["bk"], np.float32).reshape(C) + be1 @ Wk2
    bq_eff = (np.asarray(inputs["bq"], np.float32).reshape(C) * scale
              + be1 @ Wq2)
    bv_eff = np.asarray(inputs["bv"], np.float32).reshape(C) + be1 @ Wv2
    bo_eff = np.asarray(inputs["bo"], np.float32) + bv_eff @ Wo
    bos = bo_eff + be1                      # residual: + beta1 + bo
    b1_eff = np.asarray(inputs["b1"], np.float32) + be2 @ W1
    b2s = np.asarray(inputs["b2"], np.float32) + be2   # + beta2 + b2

    wk_eff = Wk2 * g1[:, None]
    wq_eff = Wq2 * g1[:, None]
    wv_eff = Wv2 * g1[:, None]
    w1_eff = W1 * g2[:, None]

    def chunk4(W, n_out):  # [C, n_out*128] -> [128p, n_out, C//128, 128]
        return np.ascontiguousarray(
            W.reshape(W.shape[0] // P, P, n_out, P).transpose(1, 2, 0, 3)
        ).astype(bf16)

    shared = {
        "wk": chunk4(wk_eff, CC),
        "wq": chunk4(wq_eff, CC),
        "wv": np.ascontiguousarray(
            wv_eff.reshape(CC, P, C).transpose(1, 0, 2)).astype(bf16),
        "wo": chunk4(Wo, CC),
        "w1": chunk4(w1_eff, MM),
        "w2": chunk4(W2, CC),
        "bq": bq_eff.reshape(CC, P).copy(),
        "bk": bk_eff.reshape(CC, P).copy(),
        "bos": bos.reshape(CC, P).copy(),
        "b1": b1_eff.reshape(MM, P).copy(),
        "b2s": b2s.reshape(CC, P).copy(),
        "g1": g1.reshape(CC, P).copy(),
        "g2": g2.reshape(CC, P).copy(),
    }
    return shared


def _core_order(j):
    return [j, 7 - j] + sorted(set(range(8)) - {j, 7 - j})


def _core_masks(j):
    """[128, 16, 256] bf16. Slots 0..7: early-half (query block j) masks for
    the BOTH chunks; slots 8..15: late-half (query block 7-j) masks for the
    LATE chunks. Chunk kc holds permuted keys order[kc//2]*256+(kc%2)*128+p."""
    order = _core_order(j)
    out = np.zeros((P, 16, QB), np.float32)
    for s, kc in enumerate(BOTH):
        key = order[kc // 2] * QB + (kc % 2) * P + np.arange(P)[:, None]
        qglob = j * QB + np.arange(QB)[None, :]
        out[:, s, :] = (key <= qglob)
    for s, kc in enumerate(LATE):
        key = order[kc // 2] * QB + (kc % 2) * P + np.arange(P)[:, None]
        qglob = (7 - j) * QB + np.arange(QB)[None, :]
        out[:, 8 + s, :] = (key <= qglob)
    return out.astype(bf16)


def _make_in_maps(inputs):
    x = np.asarray(inputs["x"], np.float32)
    shared = _prep_shared(inputs)
    in_maps = []
    for c in range(8):
        gg, j = c // 4, c % 4
        xb = x[gg]
        order = _core_order(j)
        m = dict(shared)
        m["xc"] = np.ascontiguousarray(
            np.concatenate([xb[b * QB:(b + 1) * QB] for b in order], 0))
        m["masks"] = _core_masks(j)
        in_maps.append(m)
    return in_maps


def _assemble(results):
    out = np.zeros((B, T, C), np.float32)
    for c in range(8):
        gg, j = c // 4, c % 4
        o = results[c]["out"].reshape(C, R).T  # [512, C] rows = 2 blocks
        out[gg, j * QB:(j + 1) * QB] = o[:QB]
        out[gg, (7 - j) * QB:(8 - j) * QB] = o[QB:]
    return out


def kernel(**inputs):
    in_maps = _make_in_maps(inputs)
    nc = _get_nc()
    res = run_bass_kernel_spmd(nc, in_maps, core_ids=list(range(8)))
    return _assemble(res.results)


# revision 48
# speedup vs baseline: 1.0165x; 1.0165x over previous
"""Trainium2 Bass kernel for a dense transformer block (B=2, T=2048, C=1024,
H=16, DFF=4096), distributed over 8 NeuronCores.

Sharding: 2 batch groups x 4-way query-block sharding. Core c handles batch
g=c//4 and query blocks {j, 7-j} (j=c%4). K/V are computed per-core for the
full batch (replicated; no collectives), with the batch rows PERMUTED per
core so the core's own query blocks come first: xc = [block j, block 7-j,
remaining blocks ascending]. Softmax/AV are permutation-invariant over keys,
so only the (per-core input) masks need to know the order. This makes the
first 512 rows simultaneously the query rows: LN1 runs once over 2048 rows
(no duplicated query pass), Q projects from x1T[0], and the attention
residual reads x1T[0] directly.

LayerNorm gammas/betas are folded host-side into the consuming weights
(W' = gamma*W row-scaled, b' = b + beta@W), so the device only produces the
plain normalized x-hat; the two residual adds apply gamma/beta feature-major
where they are per-partition scalars. Transposes run in bf16 (1 cyc/row),
batched 8-to-a-psum-bank with a single copy out. Scores are computed in
2-bank rounds with one exp per round; softmax denominators ride the AV
matmul as a ones-column of V; the reciprocal is broadcast on GPSIMD.

Causality at 256-block granularity: with the permuted key order, chunk pairs
{0,1},{4..9} feed both query halves (one N=512 matmul), {2,3},{10..15} feed
only the late half. Per-core 0/1/triangular masks (inputs) make one NEFF
serve all 8 cores (SPMD).
"""
import numpy as np
import ml_dtypes

import concourse.bass as bass
import concourse.mybir as mybir
import concourse.tile as tile
from concourse.vector_clock import ScopedClock
from concourse.bass_utils import run_bass_kernel_spmd
from concourse.masks import make_identity

bf16 = ml_dtypes.bfloat16
f32 = mybir.dt.float32
bt16 = mybir.dt.bfloat16
AF = mybir.ActivationFunctionType
OP = mybir.AluOpType

B, T, C, H, DH, DFF = 2, 2048, 1024, 16, 64, 4096
P = 128
QB = 256            # rows per query block
R = 512             # own query rows per core
CC = C // P         # 8 feature chunks
MM = DFF // P       # 32 ffn chunks
EPS = 1e-5

# score chunk index sets (permuted key order, see module docstring)
BOTH = (0, 1, 4, 5, 6, 7, 8, 9)           # N=512: both query halves
LATE = (2, 3, 10, 11, 12, 13, 14, 15)     # N=256: late half only
BOTH_R = ((0, 1), (4, 5), (6, 7), (8, 9))
LATE_R = ((2, 3, 10, 11), (12, 13, 14, 15))


# ---------------------------------------------------------------------------
# The walrus build in this container rejects instructions with >1 sync wait.
# Tile's sem assignment can emit several on one instruction; split the excess
# onto same-engine NoOps placed immediately before.
def _patched_drain_and_barrier(self, tick_clock, wait_clock):
    nc = self.nc
    probe = nc.sync.nop(nofuse=True, hint="tail_wait_probe")
    wait_clock.add_sem_waits(probe.ins, ScopedClock({None: tick_clock.global_clock}))
    si = probe.ins.sync_info
    waits = list(si.on_wait) if si is not None else []
    if si is not None:
        si.on_wait = waits[:1]
    for w in waits[1:]:
        n2 = nc.sync.nop(nofuse=True, hint="tail_wait_split")
        n2.ins.sync_info = mybir.SyncInfo(on_wait=[w], on_update=[])
    nc.sync.drain()
    nc.all_engine_barrier()
    assert self.sems is not None
    popped = nc._tile_sem_poison_stack.pop()
    assert popped is self._sem_poison
    nc.clear_and_free_semaphores(list(self.sems.allocated().values()))
    nc.all_engine_barrier()


tile.TileContext._drain_and_barrier = _patched_drain_and_barrier

_MAX_WAITS = 1
_split_counter = [0]


def _split_sync_waits(nc):
    for fn in nc.m.functions:
        for bb in fn.blocks:
            new_insts = []
            for inst in bb.instructions:
                si = getattr(inst, "sync_info", None)
                lim = _MAX_WAITS
                if si is not None and si.on_wait and len(si.on_wait) > lim:
                    waits = list(si.on_wait)
                    keep = waits[-lim:]
                    excess = waits[:-lim]
                    for i in range(0, len(excess), _MAX_WAITS):
                        _split_counter[0] += 1
                        nop = mybir.InstNoOp(
                            name=f"I-wsplit-{_split_counter[0]}", ins=[], outs=[])
                        nop.engine = inst.engine
                        nop.sync_info = mybir.SyncInfo(
                            on_wait=excess[i:i + _MAX_WAITS], on_update=[])
                        new_insts.append(nop)
                    si.on_wait = keep
                new_insts.append(inst)
            bb.instructions = new_insts
# ---------------------------------------------------------------------------


class Ctx:
    pass


def _ln_stats(g, xt, n_feat):
    """bn stats for one row-major tile; returns (mean[P,1], rstd[P,1])."""
    nc = g.nc
    st = g.stats.tile([P, 2, 6], f32, tag="bnst", name="bnst")
    xv = xt.rearrange("p (s d) -> p s d", s=2)
    for sg in range(2):
        nc.vector.bn_stats(out=st[:, sg, :], in_=xv[:, sg, :])
    mv = g.stats.tile([P, 2], f32, tag="bnmv", name="bnmv")
    nc.vector.bn_aggr(out=mv[:], in_=st[:])
    sq = g.stats.tile([P, 1], f32, tag="bnsq", name="bnsq")
    nc.scalar.activation(out=sq[:], in_=mv[:, 1:2], func=AF.Sqrt,
                         bias=g.eps_sb[:], scale=float(n_feat) / (n_feat - 1))
    rstd = g.stats.tile([P, 1], f32, tag="bnrstd", name="bnrstd")
    nc.vector.reciprocal(rstd[:], sq[:])
    return mv, rstd


def _ln_tile_to_fm(g, xt, out_writes, tag="xn"):
    """LN a row-major [P, C] tile to x-hat, transpose to feature-major via 8
    bf16 transposes into one psum bank, and hand the [P, CC, P] psum view to
    out_writes for the single batched copy out."""
    nc = g.nc
    mv, rstd = _ln_stats(g, xt, C)
    xn = g.xnp.tile([P, C], bt16, tag=tag, name=tag)
    nc.vector.tensor_scalar(out=xn[:], in0=xt[:], scalar1=mv[:, 0:1],
                            scalar2=rstd[:], op0=OP.subtract, op1=OP.mult)
    pt = g.ps.tile([P, C], bt16, tag="pav", name="pt_t")
    for c in range(CC):
        nc.tensor.transpose(pt[:, c * P:(c + 1) * P], xn[:, c * P:(c + 1) * P],
                            g.ident[:])
    out_writes(pt.rearrange("p (c q) -> p c q", c=CC))


def _phase_a(g):
    """LN1 over the (permuted) batch + V per row-block."""
    nc = g.nc
    for kt in range(T // P):
        nc.vector.memset(g.vv[kt][:, :, DH:DH + 1], 1.0)
    for rt in range(T // P):
        rb, r0 = rt // 4, (rt % 4) * P
        if rt < len(g.xt_pre):
            xt = g.xt_pre[rt]
        else:
            xt = g.xio.tile([P, C], f32, tag="xin", name="xin")
            nc.sync.dma_start(xt[:], g.xc[rt * P:(rt + 1) * P, :])

        def wr1(pv, rb=rb, r0=r0):
            nc.scalar.copy(out=g.x1T[rb][:, :, r0:r0 + P], in_=pv)
        _ln_tile_to_fm(g, xt, wr1)

        # V for the previous tile's keys (needs only its 128 x1T columns);
        # lagging one tile keeps PE off the Act-copy critical path
        if rt >= 1:
            _v_tile(g, rt - 1)
    _v_tile(g, T // P - 1)


def _v_tile(g, kt):
    nc = g.nc
    rb, k0 = kt // 4, (kt % 4) * P
    for half in range(2):
        pv = g.ps.tile([P, 512], f32, tag="pp", name="ps_v")
        for c in range(CC):
            nc.tensor.matmul(
                pv[:], g.x1T[rb][:, c, k0:k0 + P],
                g.wvs[:, c, half * 512:(half + 1) * 512],
                start=(c == 0), stop=(c == CC - 1))
        nc.vector.tensor_copy(
            out=g.vv[kt][:, half * 8:(half + 1) * 8, 0:DH],
            in_=pv.rearrange("p (h d) -> p h d", h=8))


def _kq_chains(g, m):
    """Issue wk/wq DMAs for pair m; return (kT, qT, [chain closures]).
    Each closure issues one 8-matmul projection chain (PE filler work)."""
    nc = g.nc
    wkm = g.wstr.tile([P, CC, P], bt16, tag="wkm", name="wkm")
    nc.sync.dma_start(wkm[:], g.wk[:, m])
    wqm = g.wstr.tile([P, CC, P], bt16, tag="wqm", name="wqm")
    nc.sync.dma_start(wqm[:], g.wq[:, m])
    kT = g.kvq.tile([P, 4, 512], bt16, tag="kT", name="kT")
    qT = g.kvq.tile([P, 512], bt16, tag="qT", name="qT")

    def k_chain(rb):
        pk = g.ps.tile([P, 512], f32, tag="pp", name="ps_k")
        for c in range(CC):
            nc.tensor.matmul(pk[:], wkm[:, c, :], g.x1T[rb][:, c, :],
                             start=(c == 0), stop=(c == CC - 1))
        nc.vector.tensor_scalar(
            out=kT[:, rb, :], in0=pk[:],
            scalar1=g.vec["bk"][:, m:m + 1], scalar2=None, op0=OP.add)

    def q_chain():
        pq = g.ps.tile([P, 512], f32, tag="pp", name="ps_q")
        for c in range(CC):
            nc.tensor.matmul(pq[:], wqm[:, c, :], g.x1T[0][:, c, :],
                             start=(c == 0), stop=(c == CC - 1))
        nc.scalar.activation(out=qT[:], in_=pq[:], func=AF.Identity,
                             bias=g.vec["bq"][:, m:m + 1], scale=1.0)

    chains = [lambda rb=rb: k_chain(rb) for rb in range(4)] + [q_chain]
    return kT, qT, chains


def _attn_head(g, kT, qT, m, hl, filler, deferred_norm):
    """Scores + exp + mask + AV for head h=2m+hl, software-pipelined at round
    granularity: AV of round r-1 issues after scores of round r, `filler`
    (next pair's K/Q chains) plugs PE stalls, and the previous head's softmax
    normalization (latency-bound) runs mid-head via `deferred_norm`."""
    nc = g.nc
    h = 2 * m + hl
    hs = slice(hl * DH, (hl + 1) * DH)
    rounds = []
    pav = g.ps.tile([P, 512], f32, tag="pav", name="ps_av")

    def score_round(r):
        psc = g.ps.tile([P, 2, 512], f32, tag="psc", name="ps_s")
        if r < 4:
            for i, kc in enumerate(BOTH_R[r]):
                rb, k0 = kc // 4, (kc % 4) * P
                nc.tensor.matmul(psc[:, i, :], kT[hs, rb, k0:k0 + P],
                                 qT[hs, :], start=True, stop=True,
                                 tile_position=(hl * DH, 0))
            aA = g.arnd.tile([P, 2, 512], bt16, tag="arnd", name="aA")
            nc.scalar.activation(out=aA[:], in_=psc[:], func=AF.Exp)
            nc.vector.tensor_mul(aA[:, :, 0:QB], aA[:, :, 0:QB],
                                 g.mq[:, 2 * r:2 * r + 2, :])
        else:
            for i, kc in enumerate(LATE_R[r - 4]):
                rb, k0 = kc // 4, (kc % 4) * P
                nc.tensor.matmul(
                    psc[:, i // 2, (i % 2) * QB:(i % 2 + 1) * QB],
                    kT[hs, rb, k0:k0 + P], qT[hs, QB:512],
                    start=True, stop=True, tile_position=(hl * DH, 0))
            aA = g.arnd.tile([P, 2, 512], bt16, tag="arnd", name="aB")
            nc.scalar.activation(out=aA[:], in_=psc[:], func=AF.Exp)
            ab4 = aA.rearrange("p a (b q) -> p (a b) q", b=2)
            nc.vector.tensor_mul(ab4[:], ab4[:],
                                 g.mq[:, 4 * r - 8:4 * r - 4, :])
        rounds.append(aA)

    def av_round(r):
        if r < 4:
            for i, kc in enumerate(BOTH_R[r]):
                nc.tensor.matmul(pav[:DH + 1, :], g.vv[kc][:, h, :],
                                 rounds[r][:, i, :], start=(r == 0 and i == 0),
                                 stop=False)
        else:
            for i, kc in enumerate(LATE_R[r - 4]):
                last = (r == 5) and (i == 3)
                nc.tensor.matmul(
                    pav[:DH + 1, QB:512], g.vv[kc][:, h, :],
                    rounds[r][:, i // 2, (i % 2) * QB:(i % 2 + 1) * QB],
                    start=False, stop=last)

    for r in range(6):
        score_round(r)
        if r == 1:
            deferred_norm()       # previous head's normalization
        if r >= 1:
            av_round(r - 1)
            filler()
    av_round(5)
    filler()

    def norm():
        rr = g.stats.tile([1, 512], bt16, tag="rr", name="rr")
        with nc.allow_low_precision(reason="1/den in bf16 is enough"):
            nc.vector.reciprocal(rr[:], pav[DH:DH + 1, :])
        pr = g.ps.tile([P, 2, 512], f32, tag="psc", name="ps_r")
        nc.tensor.matmul(pr[:DH, 0, :], g.ones64[:], rr[:],
                         start=True, stop=True)
        rbc = g.rbcp.tile([DH, 512], bt16, tag="rbc", name="rbc")
        nc.vector.tensor_copy(out=rbc[:], in_=pr[:DH, 0, :])
        nc.vector.tensor_mul(out=g.hcat[m][hs, :], in0=pav[:DH, :],
                             in1=rbc[:])
    return norm


def _phase_c(g):
    """Wo + residual -> x2T (bf16), with LN2 stats accumulated feature-major
    on the fly (column sums of x2 and x2^2 via ones-vector matmuls), then
    x3h = (x2 - mu) * rstd via broadcasted [1,512] stats. No transposes."""
    nc = g.nc
    wtiles = []
    for mo in range(3):
        wos = g.wstr.tile([P, CC, P], bt16, tag="wos", name="wos", bufs=3)
        nc.sync.dma_start(wos[:], g.wo[:, mo])
        wtiles.append(wos)
    pstat = g.ps.tile([P, 2, 512], f32, tag="psc", name="pstat")
    for mo in range(CC):
        wos = wtiles[mo]
        if mo + 3 < CC:
            nw = g.wstr.tile([P, CC, P], bt16, tag="wos", name="wos", bufs=3)
            nc.sync.dma_start(nw[:], g.wo[:, mo + 3])
            wtiles.append(nw)
        pa = g.ps.tile([P, 512], f32, tag="pp", name="ps_o")
        for c in range(CC):
            nc.tensor.matmul(pa[:], wos[:, c, :], g.hcat[c][:, :],
                             start=(c == 0), stop=(c == CC - 1))
        # x2 = gamma1 * xhat1 + (attn + bo + beta1)
        t = g.x2p.tile([P, 512], f32, tag="x2t", name="x2t")
        nc.vector.scalar_tensor_tensor(
            out=t[:], in0=g.x1T[0][:, mo, :], scalar=g.vec["g1"][:, mo:mo + 1],
            in1=pa[:], op0=OP.mult, op1=OP.add)
        nc.vector.tensor_scalar(
            out=g.x2T[mo][:], in0=t[:], scalar1=g.vec["bos"][:, mo:mo + 1],
            scalar2=None, op0=OP.add)
        sq = g.x2p.tile([P, 512], bt16, tag="sqt", name="sqt")
        nc.scalar.square(out=sq[:], in_=g.x2T[mo][:])
        nc.tensor.matmul(pstat[0:1, 0, :], g.ones128[:], g.x2T[mo][:],
                         start=(mo == 0), stop=(mo == CC - 1))
        nc.tensor.matmul(pstat[0:1, 1, :], g.ones128[:], sq[:],
                         start=(mo == 0), stop=(mo == CC - 1))
    # mu = S1/C ; var*(C-1) = S2 - C*mu^2 ; rstd = rsqrt(var + eps)
    mu = g.stats.tile([1, 512], f32, tag="mu", name="mu")
    nc.vector.tensor_scalar_mul(mu[:], pstat[0:1, 0, :], 1.0 / C)
    bsrc = g.stats.tile([1, 2, 512], f32, tag="bsrc", name="bsrc")
    musq = g.stats.tile([1, 512], f32, tag="musq", name="musq")
    nc.vector.tensor_mul(musq[:], mu[:], mu[:])
    var = g.stats.tile([1, 512], f32, tag="var", name="var")
    nc.vector.scalar_tensor_tensor(
        out=var[:], in0=musq[:], scalar=-float(C), in1=pstat[0:1, 1, :],
        op0=OP.mult, op1=OP.add)
    srt = g.stats.tile([1, 512], f32, tag="srt", name="srt")
    nc.scalar.activation(out=srt[:], in_=var[:], func=AF.Sqrt,
                         bias=g.eps_sb[0:1, :], scale=1.0 / (C - 1))
    nc.vector.reciprocal(bsrc[:, 0, :], srt[:])
    nc.vector.tensor_mul(bsrc[:, 1, :], mu[:], bsrc[:, 0, :])
    # broadcast rstd and mu*rstd to all partitions
    pb = g.ps.tile([P, 2, 512], f32, tag="psc", name="pb")
    nc.tensor.matmul(pb[:, 0, :], g.onesPf[:], bsrc[:, 0, :],
                     start=True, stop=True)
    nc.tensor.matmul(pb[:, 1, :], g.onesPf[:], bsrc[:, 1, :],
                     start=True, stop=True)
    nmsb = g.x2p.tile([P, 512], f32, tag="nmsb", name="nmsb")
    nc.scalar.copy(out=nmsb[:], in_=pb[:, 1, :])
    for c in range(CC):
        t1 = g.x2p.tile([P, 512], f32, tag="x2t", name="x3t")
        nc.vector.tensor_mul(t1[:], g.x2T[c][:], pb[:, 0, :])
        # subtract on GPSIMD so W1's first chain is not DVE-paced
        nc.gpsimd.tensor_sub(g.x3h[:, c, :], t1[:], nmsb[:])


def _phase_d(g):
    """LN2 (to x-hat, gamma2/beta2 folded into W1/outputs) + FFN + out."""
    nc, tc = g.nc, g.tc
    with tc.tile_pool(name="dp", bufs=1) as dp, \
         tc.tile_pool(name="w1p", bufs=4) as w1p, \
         tc.tile_pool(name="outp", bufs=2) as outp, \
         tc.tile_pool(name="w2s", bufs=2) as w2s:
        g.outp, g.w2s = outp, w2s
        _phase_d_body(g, dp, w1p)


def _phase_d_body(g, dp, w1p):
    nc = g.nc
    w1tiles = []
    for mm in range(3):
        w1m = w1p.tile([P, CC, P], bt16, tag="w1m", name="w1m")
        nc.sync.dma_start(w1m[:], g.w1[:, mm])
        w1tiles.append(w1m)

    h1 = [dp.tile([P, 512], bt16, tag=f"h1_{mm}", name=f"h1_{mm}")
          for mm in range(MM)]
    for mm in range(MM):
        w1m = w1tiles[mm]
        if mm + 3 < MM:
            nw = w1p.tile([P, CC, P], bt16, tag="w1m", name="w1m")
            nc.sync.dma_start(nw[:], g.w1[:, mm + 3])
            w1tiles.append(nw)
        if mm == MM - 2:
            # prefetch the first W2 chunks behind the last W1 loads
            w2tiles = []
            for oc in range(2):
                w2m = g.w2s.tile([P, MM, P], bt16, tag="w2m", name="w2m")
                nc.sync.dma_start(w2m[:], g.w2[:, oc])
                w2tiles.append(w2m)
        p1 = g.ps.tile([P, 512], f32, tag="pp", name="ps_f1")
        for c in range(CC):
            nc.tensor.matmul(p1[:], w1m[:, c, :], g.x3h[:, c, :],
                             start=(c == 0), stop=(c == CC - 1))
        nc.scalar.activation(out=h1[mm][:], in_=p1[:], func=AF.Gelu,
                             bias=g.vec["b1"][:, mm:mm + 1], scale=1.0)
    for oc in range(CC):
        w2m = w2tiles[oc]
        if oc + 2 < CC:
            nw = g.w2s.tile([P, MM, P], bt16, tag="w2m", name="w2m")
            nc.sync.dma_start(nw[:], g.w2[:, oc + 2])
            w2tiles.append(nw)
        p2 = g.ps.tile([P, 512], f32, tag="pp", name="ps_f2")
        for k in range(MM):
            nc.tensor.matmul(p2[:], w2m[:, k, :], h1[k][:, :],
                             start=(k == 0), stop=(k == MM - 1))
        # out = gamma2 * xhat2 + (ffw + b2 + beta2)
        t = g.x2p.tile([P, 512], f32, tag="x2t", name="x2o")
        nc.vector.scalar_tensor_tensor(
            out=t[:], in0=g.x3h[:, oc, :], scalar=g.vec["g2"][:, oc:oc + 1],
            in1=p2[:], op0=OP.mult, op1=OP.add)
        ot = g.outp.tile([P, 512], f32, tag="otile", name="otile")
        nc.vector.tensor_scalar(
            out=ot[:], in0=t[:], scalar1=g.vec["b2s"][:, oc:oc + 1],
            scalar2=None, op0=OP.add)
        nc.sync.dma_start(g.out[oc], ot[:])


def build_kernel():
    nc = bass.Bass("TRN2", target_bir_lowering=False, num_devices=8)
    g = Ctx()
    g.nc = nc

    g.xc = nc.dram_tensor("xc", [T, C], f32, kind="ExternalInput").ap()
    g.wk = nc.dram_tensor("wk", [P, CC, CC, P], bt16, kind="ExternalInput").ap()
    g.wq = nc.dram_tensor("wq", [P, CC, CC, P], bt16, kind="ExternalInput").ap()
    g.wv = nc.dram_tensor("wv", [P, CC, C], bt16, kind="ExternalInput").ap()
    g.wo = nc.dram_tensor("wo", [P, CC, CC, P], bt16, kind="ExternalInput").ap()
    g.w1 = nc.dram_tensor("w1", [P, MM, CC, P], bt16, kind="ExternalInput").ap()
    g.w2 = nc.dram_tensor("w2", [P, CC, MM, P], bt16, kind="ExternalInput").ap()
    g.masks = nc.dram_tensor("masks", [P, 16, QB], bt16,
                             kind="ExternalInput").ap()
    vecs = {}
    for nm, n in [("bq", CC), ("bk", CC), ("bos", CC), ("b1", MM),
                  ("b2s", CC), ("g1", CC), ("g2", CC)]:
        vecs[nm] = nc.dram_tensor(nm, [n, P], f32, kind="ExternalInput").ap()
    g.out = nc.dram_tensor("out", [CC, P, R], f32, kind="ExternalOutput").ap()

    with tile.TileContext(nc) as tc:
        g.tc = tc
        _build_body(g, vecs)
    _split_sync_waits(nc)
    return nc


def _build_body(g, vecs):
    nc, tc = g.nc, g.tc
    from contextlib import ExitStack
    with ExitStack() as es:
        pool = lambda name, bufs, **kw: es.enter_context(
            tc.tile_pool(name=name, bufs=bufs, **kw))
        setup = pool("setup", 1)
        g.setup = setup
        g.stats = pool("stats", 4)
        g.ps = pool("ps", 2, space="PSUM")
        g.xnp = pool("xnp", 2)
        g.rbcp = pool("rbcp", 2)
        hp = pool("hp", 1)
        g.x2p = pool("x2p", 2)
        x2Tp = pool("x2Tp", 1)
        x3p = pool("x3p", 1)

        ident = setup.tile([P, P], bt16, tag="ident", name="ident")
        make_identity(nc, ident[:])
        g.ident = ident
        g.eps_sb = setup.tile([P, 1], f32, tag="eps", name="eps")
        nc.vector.memset(g.eps_sb[:], EPS)
        g.ones64 = setup.tile([1, DH], bt16, tag="ones64", name="ones64")
        nc.vector.memset(g.ones64[:], 1.0)
        g.ones128 = setup.tile([P, 1], bt16, tag="ones128", name="ones128")
        nc.vector.memset(g.ones128[:], 1.0)
        g.onesPf = setup.tile([1, P], f32, tag="onesPf", name="onesPf")
        nc.vector.memset(g.onesPf[:], 1.0)
        # warm the PE p-state while the first x DMA is in flight
        pwarm = g.ps.tile([P, 2048], bt16, tag="psc", name="pwarm")
        for i in range(16):
            nc.tensor.transpose(pwarm[:, i * P:(i + 1) * P], ident[:],
                                ident[:])

        g.hcat = [hp.tile([P, 512], bt16, tag=f"hcat{c}", name=f"hcat{c}")
                  for c in range(CC)]
        g.x2T = [x2Tp.tile([P, 512], bt16, tag=f"x2T{mo}", name=f"x2T{mo}")
                 for mo in range(CC)]
        g.x3h = x3p.tile([P, CC, 512], bt16, tag="x3h", name="x3h")

        with ExitStack() as es2:
            pool2 = lambda name, bufs: es2.enter_context(
                tc.tile_pool(name=name, bufs=bufs))
            x1p = pool2("x1p", 1)
            kvp = pool2("kvp", 1)
            g.wstr = pool2("wstr", 2)
            g.x1T = [x1p.tile([P, CC, 512], bt16, tag=f"x1T{rb}",
                              name=f"x1T{rb}") for rb in range(4)]
            g.vv = [kvp.tile([P, H, DH + 1], bt16, tag=f"vv{kt}",
                             name=f"vv{kt}") for kt in range(T // P)]
            with ExitStack() as esa:
                poola = lambda name, bufs: esa.enter_context(
                    tc.tile_pool(name=name, bufs=bufs))
                g.xio = poola("xio", 3)
                wvp = poola("wvp", 1)
                # first xin DMAs go out before anything else
                g.xt_pre = []
                for rt in range(3):
                    xt = g.xio.tile([P, C], f32, tag="xin", name="xin")
                    nc.sync.dma_start(xt[:], g.xc[rt * P:(rt + 1) * P, :])
                    g.xt_pre.append(xt)
                g.wvs = wvp.tile([P, CC, C], bt16, tag="wvs", name="wvs")
                nc.sync.dma_start(g.wvs[:], g.wv)
                g.vec = {}
                for nm, ap_ in vecs.items():
                    n = ap_.shape[0]
                    t = g.setup.tile([P, n], f32, tag=f"vec_{nm}",
                                     name=f"vec_{nm}")
                    nc.sync.dma_start(t[:], ap_.rearrange("c p -> p c"))
                    g.vec[nm] = t
                _phase_a(g)
            with ExitStack() as esb:
                poolb = lambda name, bufs: esb.enter_context(
                    tc.tile_pool(name=name, bufs=bufs))
                g.kvq = poolb("kvq", 3)
                mp = poolb("mp", 1)
                g.arnd = poolb("arp", 6)
                g.mq = mp.tile([P, 16, QB], bt16, tag="mask", name="mask")
                nc.sync.dma_start(g.mq[:], g.masks)
                _phase_b(g)
                _phase_c(g)
        _phase_d(g)


def _phase_b(g):
    kq = {}
    pending = []          # [(pair_index, chain_closure), ...] in order

    def ensure(mi):
        if mi < CC and mi not in kq:
            kT, qT, chains = _kq_chains(g, mi)
            kq[mi] = (kT, qT)
            pending.extend((mi, ch) for ch in chains)

    def filler():
        if pending:
            pending.pop(0)[1]()

    norms = []

    def deferred_norm():
        if norms:
            norms.pop(0)()

    ensure(0)
    while pending:
        pending.pop(0)[1]()
    ensure(1)
    for m in range(CC):
        ensure(m + 2)
        kT, qT = kq[m]
        for hl in range(2):
            norms.append(_attn_head(g, kT, qT, m, hl, filler, deferred_norm))
        # everything pair m+1 references must be issued before its scores
        while pending and pending[0][0] <= m + 1:
            pending.pop(0)[1]()
    while norms:
        norms.pop(0)()


_NC_CACHE = None


def _get_nc():
    global _NC_CACHE
    if _NC_CACHE is None:
        _NC_CACHE = build_kernel()
    return _NC_CACHE


def _prep_shared(inputs):
    scale = DH ** -0.5
    Wq = np.asarray(inputs["Wq"], np.float32)
    Wk = np.asarray(inputs["Wk"], np.float32)
    Wv = np.asarray(inputs["Wv"], np.float32)
    Wo = np.asarray(inputs["Wo"], np.float32)
    W1 = np.asarray(inputs["W1"], np.float32)
    W2 = np.asarray(inputs["W2"], np.float32)
    g1 = np.asarray(inputs["gamma1"], np.float32)
    be1 = np.asarray(inputs["beta1"], np.float32)
    g2 = np.asarray(inputs["gamma2"], np.float32)
    be2 = np.asarray(inputs["beta2"], np.float32)

    Wk2 = np.ascontiguousarray(Wk.transpose(1, 0, 2).reshape(C, C))
    Wq2 = np.ascontiguousarray(Wq.transpose(1, 0, 2).reshape(C, C)) * scale
    Wv2 = np.ascontiguousarray(Wv.transpose(1, 0, 2).reshape(C, C))

    bk_eff = np.asarray(inputs["bk"], np.float32).reshape(C) + be1 @ Wk2
    bq_eff = (np.asarray(inputs["bq"], np.float32).reshape(C) * scale
              + be1 @ Wq2)
    bv_eff = np.asarray(inputs["bv"], np.float32).reshape(C) + be1 @ Wv2
    bo_eff = np.asarray(inputs["bo"], np.float32) + bv_eff @ Wo
    bos = bo_eff + be1                      # residual: + beta1 + bo
    b1_eff = np.asarray(inputs["b1"], np.float32) + be2 @ W1
    b2s = np.asarray(inputs["b2"], np.float32) + be2   # + beta2 + b2

    wk_eff = Wk2 * g1[:, None]
    wq_eff = Wq2 * g1[:, None]
    wv_eff = Wv2 * g1[:, None]
    w1_eff = W1 * g2[:, None]

    def chunk4(W, n_out):  # [C, n_out*128] -> [128p, n_out, C//128, 128]
        return np.ascontiguousarray(
            W.reshape(W.shape[0] // P, P, n_out, P).transpose(1, 2, 0, 3)
        ).astype(bf16)

    shared = {
        "wk": chunk4(wk_eff, CC),
        "wq": chunk4(wq_eff, CC),
        "wv": np.ascontiguousarray(
            wv_eff.reshape(CC, P, C).transpose(1, 0, 2)).astype(bf16),
        "wo": chunk4(Wo, CC),
        "w1": chunk4(w1_eff, MM),
        "w2": chunk4(W2, CC),
        "bq": bq_eff.reshape(CC, P).copy(),
        "bk": bk_eff.reshape(CC, P).copy(),
        "bos": bos.reshape(CC, P).copy(),
        "b1": b1_eff.reshape(MM, P).copy(),
        "b2s": b2s.reshape(CC, P).copy(),
        "g1": g1.reshape(CC, P).copy(),
        "g2": g2.reshape(CC, P).copy(),
    }
    return shared


def _core_order(j):
    return [j, 7 - j] + sorted(set(range(8)) - {j, 7 - j})


def _core_masks(j):
    """[128, 16, 256] bf16. Slots 0..7: early-half (query block j) masks for
    the BOTH chunks; slots 8..15: late-half (query block 7-j) masks for the
    LATE chunks. Chunk kc holds permuted keys order[kc//2]*256+(kc%2)*128+p."""
    order = _core_order(j)
    out = np.zeros((P, 16, QB), np.float32)
    for s, kc in enumerate(BOTH):
        key = order[kc // 2] * QB + (kc % 2) * P + np.arange(P)[:, None]
        qglob = j * QB + np.arange(QB)[None, :]
        out[:, s, :] = (key <= qglob)
    for s, kc in enumerate(LATE):
        key = order[kc // 2] * QB + (kc % 2) * P + np.arange(P)[:, None]
        qglob = (7 - j) * QB + np.arange(QB)[None, :]
        out[:, 8 + s, :] = (key <= qglob)
    return out.astype(bf16)


def _make_in_maps(inputs):
    x = np.asarray(inputs["x"], np.float32)
    shared = _prep_shared(inputs)
    in_maps = []
    for c in range(8):
        gg, j = c // 4, c % 4
        xb = x[gg]
        order = _core_order(j)
        m = dict(shared)
        m["xc"] = np.ascontiguousarray(
            np.concatenate([xb[b * QB:(b + 1) * QB] for b in order], 0))
        m["masks"] = _core_masks(j)
        in_maps.append(m)
    return in_maps


def _assemble(results):
    out = np.zeros((B, T, C), np.float32)
    for c in range(8):
        gg, j = c // 4, c % 4
        o = results[c]["out"].reshape(C, R).T  # [512, C] rows = 2 blocks
        out[gg, j * QB:(j + 1) * QB] = o[:QB]
        out[gg, (7 - j) * QB:(8 - j) * QB] = o[QB:]
    return out


def kernel(**inputs):
    in_maps = _make_in_maps(inputs)
    nc = _get_nc()
    res = run_bass_kernel_spmd(nc, in_maps, core_ids=list(range(8)))
    return _assemble(res.results)


# revision 55
# speedup vs baseline: 1.0201x; 1.0036x over previous
"""Trainium2 Bass kernel for a dense transformer block (B=2, T=2048, C=1024,
H=16, DFF=4096), distributed over 8 NeuronCores.

Sharding: 2 batch groups x 4-way query-block sharding. Core c handles batch
g=c//4 and query blocks {j, 7-j} (j=c%4). K/V are computed per-core for the
full batch (replicated; no collectives), with the batch rows PERMUTED per
core so the core's own query blocks come first: xc = [block j, block 7-j,
remaining blocks ascending]. Softmax/AV are permutation-invariant over keys,
so only the (per-core input) masks need to know the order. This makes the
first 512 rows simultaneously the query rows: LN1 runs once over 2048 rows
(no duplicated query pass), Q projects from x1T[0], and the attention
residual reads x1T[0] directly.

LayerNorm gammas/betas are folded host-side into the consuming weights
(W' = gamma*W row-scaled, b' = b + beta@W), so the device only produces the
plain normalized x-hat; the two residual adds apply gamma/beta feature-major
where they are per-partition scalars. Transposes run in bf16 (1 cyc/row),
batched 8-to-a-psum-bank with a single copy out. Scores are computed in
2-bank rounds with one exp per round; softmax denominators ride the AV
matmul as a ones-column of V; the reciprocal is broadcast on GPSIMD.

Causality at 256-block granularity: with the permuted key order, chunk pairs
{0,1},{4..9} feed both query halves (one N=512 matmul), {2,3},{10..15} feed
only the late half. Per-core 0/1/triangular masks (inputs) make one NEFF
serve all 8 cores (SPMD).
"""
import numpy as np
import ml_dtypes

import concourse.bass as bass
import concourse.mybir as mybir
import concourse.tile as tile
from concourse.vector_clock import ScopedClock
from concourse.bass_utils import run_bass_kernel_spmd
from concourse.masks import make_identity

bf16 = ml_dtypes.bfloat16
f32 = mybir.dt.float32
bt16 = mybir.dt.bfloat16
AF = mybir.ActivationFunctionType
OP = mybir.AluOpType

B, T, C, H, DH, DFF = 2, 2048, 1024, 16, 64, 4096
P = 128
QB = 256            # rows per query block
R = 512             # own query rows per core
CC = C // P         # 8 feature chunks
MM = DFF // P       # 32 ffn chunks
EPS = 1e-5

# score chunk index sets (permuted key order, see module docstring)
BOTH = (0, 1, 4, 5, 6, 7, 8, 9)           # N=512: both query halves
LATE = (2, 3, 10, 11, 12, 13, 14, 15)     # N=256: late half only
BOTH_R = ((0, 1), (4, 5), (6, 7), (8, 9))
LATE_R = ((2, 3, 10, 11), (12, 13, 14, 15))


# ---------------------------------------------------------------------------
# The walrus build in this container rejects instructions with >1 sync wait.
# Tile's sem assignment can emit several on one instruction; split the excess
# onto same-engine NoOps placed immediately before.
def _patched_drain_and_barrier(self, tick_clock, wait_clock):
    nc = self.nc
    probe = nc.sync.nop(nofuse=True, hint="tail_wait_probe")
    wait_clock.add_sem_waits(probe.ins, ScopedClock({None: tick_clock.global_clock}))
    si = probe.ins.sync_info
    waits = list(si.on_wait) if si is not None else []
    if si is not None:
        si.on_wait = waits[:1]
    for w in waits[1:]:
        n2 = nc.sync.nop(nofuse=True, hint="tail_wait_split")
        n2.ins.sync_info = mybir.SyncInfo(on_wait=[w], on_update=[])
    nc.sync.drain()
    nc.all_engine_barrier()
    assert self.sems is not None
    popped = nc._tile_sem_poison_stack.pop()
    assert popped is self._sem_poison
    nc.clear_and_free_semaphores(list(self.sems.allocated().values()))
    nc.all_engine_barrier()


tile.TileContext._drain_and_barrier = _patched_drain_and_barrier

_MAX_WAITS = 1
_split_counter = [0]


def _split_sync_waits(nc):
    for fn in nc.m.functions:
        for bb in fn.blocks:
            new_insts = []
            for inst in bb.instructions:
                si = getattr(inst, "sync_info", None)
                lim = _MAX_WAITS
                if si is not None and si.on_wait and len(si.on_wait) > lim:
                    waits = list(si.on_wait)
                    keep = waits[-lim:]
                    excess = waits[:-lim]
                    for i in range(0, len(excess), _MAX_WAITS):
                        _split_counter[0] += 1
                        nop = mybir.InstNoOp(
                            name=f"I-wsplit-{_split_counter[0]}", ins=[], outs=[])
                        nop.engine = inst.engine
                        nop.sync_info = mybir.SyncInfo(
                            on_wait=excess[i:i + _MAX_WAITS], on_update=[])
                        new_insts.append(nop)
                    si.on_wait = keep
                new_insts.append(inst)
            bb.instructions = new_insts
# ---------------------------------------------------------------------------


class Ctx:
    pass


def _ln_stats(g, xt, n_feat):
    """bn stats for one row-major tile; returns (mean[P,1], rstd[P,1])."""
    nc = g.nc
    st = g.stats.tile([P, 2, 6], f32, tag="bnst", name="bnst")
    xv = xt.rearrange("p (s d) -> p s d", s=2)
    for sg in range(2):
        nc.vector.bn_stats(out=st[:, sg, :], in_=xv[:, sg, :])
    mv = g.stats.tile([P, 2], f32, tag="bnmv", name="bnmv")
    nc.vector.bn_aggr(out=mv[:], in_=st[:])
    sq = g.stats.tile([P, 1], f32, tag="bnsq", name="bnsq")
    nc.scalar.activation(out=sq[:], in_=mv[:, 1:2], func=AF.Sqrt,
                         bias=g.eps_sb[:], scale=float(n_feat) / (n_feat - 1))
    rstd = g.stats.tile([P, 1], f32, tag="bnrstd", name="bnrstd")
    nc.vector.reciprocal(rstd[:], sq[:])
    return mv, rstd


def _ln_tile_to_fm(g, xt, out_writes, tag="xn"):
    """LN a row-major [P, C] tile to x-hat, transpose to feature-major via 8
    bf16 transposes into one psum bank, and hand the [P, CC, P] psum view to
    out_writes for the single batched copy out."""
    nc = g.nc
    mv, rstd = _ln_stats(g, xt, C)
    xn = g.xnp.tile([P, C], bt16, tag=tag, name=tag)
    nc.vector.tensor_scalar(out=xn[:], in0=xt[:], scalar1=mv[:, 0:1],
                            scalar2=rstd[:], op0=OP.subtract, op1=OP.mult)
    pt = g.ps.tile([P, C], bt16, tag="pav", name="pt_t")
    for c in range(CC):
        nc.tensor.transpose(pt[:, c * P:(c + 1) * P], xn[:, c * P:(c + 1) * P],
                            g.ident[:])
    out_writes(pt.rearrange("p (c q) -> p c q", c=CC))


def _xin_dma(g, xt, rt):
    """Load one x row-tile in two half-column DMAs so bn_stats on the first
    half can start while the second half is still in flight."""
    nc = g.nc
    for h in range(2):
        nc.sync.dma_start(xt[:, h * 512:(h + 1) * 512],
                          g.xc[rt * P:(rt + 1) * P, h * 512:(h + 1) * 512])


def _phase_a(g):
    """LN1 over the (permuted) batch + V per row-block."""
    nc = g.nc
    for kt in range(T // P):
        nc.vector.memset(g.vv[kt][:, :, DH:DH + 1], 1.0)
    for rt in range(T // P):
        rb, r0 = rt // 4, (rt % 4) * P
        if rt < len(g.xt_pre):
            xt = g.xt_pre[rt]
        else:
            xt = g.xio.tile([P, C], f32, tag="xin", name="xin")
            _xin_dma(g, xt, rt)

        def wr1(pv, rb=rb, r0=r0):
            nc.scalar.copy(out=g.x1T[rb][:, :, r0:r0 + P], in_=pv)
        _ln_tile_to_fm(g, xt, wr1)

        # V for the previous tile's keys (needs only its 128 x1T columns);
        # lagging one tile keeps PE off the Act-copy critical path
        if rt >= 1:
            _v_tile(g, rt - 1)
    _v_tile(g, T // P - 1)


def _v_tile(g, kt):
    nc = g.nc
    rb, k0 = kt // 4, (kt % 4) * P
    for half in range(2):
        pv = g.ps.tile([P, 512], f32, tag="pp", name="ps_v")
        for c in range(CC):
            nc.tensor.matmul(
                pv[:], g.x1T[rb][:, c, k0:k0 + P],
                g.wvs[:, c, half * 512:(half + 1) * 512],
                start=(c == 0), stop=(c == CC - 1))
        nc.vector.tensor_copy(
            out=g.vv[kt][:, half * 8:(half + 1) * 8, 0:DH],
            in_=pv.rearrange("p (h d) -> p h d", h=8))


def _kq_chains(g, m):
    """Issue wk/wq DMAs for pair m; return (kT, qT, [chain closures]).
    Each closure issues one 8-matmul projection chain (PE filler work)."""
    nc = g.nc
    wkm = g.wstr.tile([P, CC, P], bt16, tag="wkm", name="wkm")
    nc.sync.dma_start(wkm[:], g.wk[:, m])
    wqm = g.wstr.tile([P, CC, P], bt16, tag="wqm", name="wqm")
    nc.sync.dma_start(wqm[:], g.wq[:, m])
    kT = g.kvq.tile([P, 4, 512], bt16, tag="kT", name="kT")
    qT = g.kvq.tile([P, 512], bt16, tag="qT", name="qT")

    def k_chain(rb):
        pk = g.ps.tile([P, 512], f32, tag="pp", name="ps_k")
        for c in range(CC):
            nc.tensor.matmul(pk[:], wkm[:, c, :], g.x1T[rb][:, c, :],
                             start=(c == 0), stop=(c == CC - 1))
        nc.vector.tensor_scalar(
            out=kT[:, rb, :], in0=pk[:],
            scalar1=g.vec["bk"][:, m:m + 1], scalar2=None, op0=OP.add)

    def q_chain():
        pq = g.ps.tile([P, 512], f32, tag="pp", name="ps_q")
        for c in range(CC):
            nc.tensor.matmul(pq[:], wqm[:, c, :], g.x1T[0][:, c, :],
                             start=(c == 0), stop=(c == CC - 1))
        nc.scalar.activation(out=qT[:], in_=pq[:], func=AF.Identity,
                             bias=g.vec["bq"][:, m:m + 1], scale=1.0)

    chains = [lambda rb=rb: k_chain(rb) for rb in range(4)] + [q_chain]
    return kT, qT, chains


def _attn_head(g, kT, qT, m, hl, filler, deferred_norm):
    """Scores + exp + mask + AV for head h=2m+hl, software-pipelined at round
    granularity: AV of round r-1 issues after scores of round r, `filler`
    (next pair's K/Q chains) plugs PE stalls, and the previous head's softmax
    normalization (latency-bound) runs mid-head via `deferred_norm`."""
    nc = g.nc
    h = 2 * m + hl
    hs = slice(hl * DH, (hl + 1) * DH)
    rounds = []
    pav = g.ps.tile([P, 512], f32, tag="pav", name="ps_av")

    def score_round(r):
        psc = g.ps.tile([P, 2, 512], f32, tag="psc", name="ps_s")
        if r < 4:
            for i, kc in enumerate(BOTH_R[r]):
                rb, k0 = kc // 4, (kc % 4) * P
                nc.tensor.matmul(psc[:, i, :], kT[hs, rb, k0:k0 + P],
                                 qT[hs, :], start=True, stop=True,
                                 tile_position=(hl * DH, 0))
            aA = g.arnd.tile([P, 2, 512], bt16, tag="arnd", name="aA")
            nc.scalar.activation(out=aA[:], in_=psc[:], func=AF.Exp)
            nc.vector.tensor_mul(aA[:, :, 0:QB], aA[:, :, 0:QB],
                                 g.mq[:, 2 * r:2 * r + 2, :])
        else:
            for i, kc in enumerate(LATE_R[r - 4]):
                rb, k0 = kc // 4, (kc % 4) * P
                nc.tensor.matmul(
                    psc[:, i // 2, (i % 2) * QB:(i % 2 + 1) * QB],
                    kT[hs, rb, k0:k0 + P], qT[hs, QB:512],
                    start=True, stop=True, tile_position=(hl * DH, 0))
            aA = g.arnd.tile([P, 2, 512], bt16, tag="arnd", name="aB")
            nc.scalar.activation(out=aA[:], in_=psc[:], func=AF.Exp)
            ab4 = aA.rearrange("p a (b q) -> p (a b) q", b=2)
            nc.vector.tensor_mul(ab4[:], ab4[:],
                                 g.mq[:, 4 * r - 8:4 * r - 4, :])
        rounds.append(aA)

    def av_round(r):
        if r < 4:
            for i, kc in enumerate(BOTH_R[r]):
                nc.tensor.matmul(pav[:DH + 1, :], g.vv[kc][:, h, :],
                                 rounds[r][:, i, :], start=(r == 0 and i == 0),
                                 stop=False)
        else:
            for i, kc in enumerate(LATE_R[r - 4]):
                last = (r == 5) and (i == 3)
                nc.tensor.matmul(
                    pav[:DH + 1, QB:512], g.vv[kc][:, h, :],
                    rounds[r][:, i // 2, (i % 2) * QB:(i % 2 + 1) * QB],
                    start=False, stop=last)

    for r in range(6):
        score_round(r)
        if r == 1:
            deferred_norm()       # previous head's normalization
        if r >= 1:
            av_round(r - 1)
            filler()
    av_round(5)
    filler()

    def norm():
        rr = g.stats.tile([1, 512], bt16, tag="rr", name="rr")
        with nc.allow_low_precision(reason="1/den in bf16 is enough"):
            nc.vector.reciprocal(rr[:], pav[DH:DH + 1, :])
        pr = g.ps.tile([P, 2, 512], f32, tag="psc", name="ps_r")
        nc.tensor.matmul(pr[:DH, 0, :], g.ones64[:], rr[:],
                         start=True, stop=True)
        rbc = g.rbcp.tile([DH, 512], bt16, tag="rbc", name="rbc")
        nc.vector.tensor_copy(out=rbc[:], in_=pr[:DH, 0, :])
        nc.vector.tensor_mul(out=g.hcat[m][hs, :], in0=pav[:DH, :],
                             in1=rbc[:])
    return norm


def _phase_c(g):
    """Wo + residual -> x2T (bf16), with LN2 stats accumulated feature-major
    on the fly (column sums of x2 and x2^2 via ones-vector matmuls), then
    x3h = (x2 - mu) * rstd via broadcasted [1,512] stats. No transposes."""
    nc = g.nc
    wtiles = []
    for mo in range(3):
        wos = g.wstr.tile([P, CC, P], bt16, tag="wos", name="wos", bufs=3)
        nc.sync.dma_start(wos[:], g.wo[:, mo])
        wtiles.append(wos)
    pstat = g.ps.tile([P, 2, 512], f32, tag="psc", name="pstat")
    deferred_c7 = []
    for mo in range(CC):
        wos = wtiles[mo]
        if mo + 3 < CC:
            nw = g.wstr.tile([P, CC, P], bt16, tag="wos", name="wos", bufs=3)
            nc.sync.dma_start(nw[:], g.wo[:, mo + 3])
            wtiles.append(nw)
        pa = g.ps.tile([P, 512], f32, tag="pp", name="ps_o")
        if mo == 0:
            # chunks 0 and 1: issue c=0..6 for both before either touches
            # hcat7, absorbing the last head's normalization latency
            pa1 = g.ps.tile([P, 512], f32, tag="pp", name="ps_o1")
            for c in range(CC - 1):
                nc.tensor.matmul(pa[:], wos[:, c, :], g.hcat[c][:, :],
                                 start=(c == 0), stop=False)
            for c in range(CC - 1):
                nc.tensor.matmul(pa1[:], wtiles[1][:, c, :], g.hcat[c][:, :],
                                 start=(c == 0), stop=False)
            nc.tensor.matmul(pa[:], wos[:, CC - 1, :], g.hcat[CC - 1][:, :],
                             start=False, stop=True)
            nc.tensor.matmul(pa1[:], wtiles[1][:, CC - 1, :],
                             g.hcat[CC - 1][:, :], start=False, stop=True)
            deferred_c7.append(pa1)
        elif mo == 1:
            pa = deferred_c7.pop()
        else:
            for c in range(CC):
                nc.tensor.matmul(pa[:], wos[:, c, :], g.hcat[c][:, :],
                                 start=(c == 0), stop=(c == CC - 1))
        # x2 = gamma1 * xhat1 + (attn + bo + beta1)
        t = g.x2p.tile([P, 512], f32, tag="x2t", name="x2t")
        nc.vector.scalar_tensor_tensor(
            out=t[:], in0=g.x1T[0][:, mo, :], scalar=g.vec["g1"][:, mo:mo + 1],
            in1=pa[:], op0=OP.mult, op1=OP.add)
        nc.vector.tensor_scalar(
            out=g.x2T[mo][:], in0=t[:], scalar1=g.vec["bos"][:, mo:mo + 1],
            scalar2=None, op0=OP.add)
        sq = g.x2p.tile([P, 512], bt16, tag="sqt", name="sqt")
        nc.scalar.square(out=sq[:], in_=g.x2T[mo][:])
        nc.tensor.matmul(pstat[0:1, 0, :], g.ones128[:], g.x2T[mo][:],
                         start=(mo == 0), stop=(mo == CC - 1))
        nc.tensor.matmul(pstat[0:1, 1, :], g.ones128[:], sq[:],
                         start=(mo == 0), stop=(mo == CC - 1))
    # S1^2 -> var*(C-1)*C = C*S2 - S1^2 ; rstd = 1/sqrt(var + eps)
    bsrc = g.stats.tile([1, 2, 512], f32, tag="bsrc", name="bsrc")
    s1 = g.stats.tile([1, 512], f32, tag="s1", name="s1")
    nc.vector.tensor_copy(s1[:], pstat[0:1, 0, :])
    musq = g.stats.tile([1, 512], f32, tag="musq", name="musq")
    nc.vector.tensor_mul(musq[:], s1[:], s1[:])
    var = g.stats.tile([1, 512], f32, tag="var", name="var")
    nc.vector.scalar_tensor_tensor(
        out=var[:], in0=pstat[0:1, 1, :], scalar=float(C),
        in1=musq[:], op0=OP.mult, op1=OP.subtract)
    srt = g.stats.tile([1, 512], f32, tag="srt", name="srt")
    nc.scalar.activation(out=srt[:], in_=var[:], func=AF.Sqrt,
                         bias=g.eps_sb[0:1, :],
                         scale=1.0 / (float(C) * (C - 1)))
    nc.vector.reciprocal(bsrc[:, 0, :], srt[:])
    # mu*rstd = S1*rstd/C
    nc.vector.scalar_tensor_tensor(
        out=bsrc[:, 1, :], in0=s1[:], scalar=1.0 / C,
        in1=bsrc[:, 0, :], op0=OP.mult, op1=OP.mult)
    # broadcast rstd and mu*rstd to all partitions
    pb = g.ps.tile([P, 2, 512], f32, tag="psc", name="pb")
    nc.tensor.matmul(pb[:, 0, :], g.onesPf[:], bsrc[:, 0, :],
                     start=True, stop=True)
    nc.tensor.matmul(pb[:, 1, :], g.onesPf[:], bsrc[:, 1, :],
                     start=True, stop=True)
    nmsb = g.x2p.tile([P, 512], f32, tag="nmsb", name="nmsb")
    nc.scalar.copy(out=nmsb[:], in_=pb[:, 1, :])
    for c in range(CC):
        t1 = g.x2p.tile([P, 512], f32, tag="x2t", name="x3t")
        nc.vector.tensor_mul(t1[:], g.x2T[c][:], pb[:, 0, :])
        # subtract on GPSIMD so W1's first chain is not DVE-paced
        nc.gpsimd.tensor_sub(g.x3h[:, c, :], t1[:], nmsb[:])


def _phase_d(g):
    """LN2 (to x-hat, gamma2/beta2 folded into W1/outputs) + FFN + out."""
    nc, tc = g.nc, g.tc
    with tc.tile_pool(name="dp", bufs=1) as dp, \
         tc.tile_pool(name="w1p", bufs=4) as w1p, \
         tc.tile_pool(name="outp", bufs=2) as outp, \
         tc.tile_pool(name="w2s", bufs=2) as w2s:
        g.outp, g.w2s = outp, w2s
        _phase_d_body(g, dp, w1p)


def _phase_d_body(g, dp, w1p):
    nc = g.nc
    w1tiles = []
    for mm in range(3):
        w1m = w1p.tile([P, CC, P], bt16, tag="w1m", name="w1m")
        nc.sync.dma_start(w1m[:], g.w1[:, mm])
        w1tiles.append(w1m)

    h1 = [dp.tile([P, 512], bt16, tag=f"h1_{mm}", name=f"h1_{mm}")
          for mm in range(MM)]
    for mm in range(MM):
        w1m = w1tiles[mm]
        if mm + 3 < MM:
            nw = w1p.tile([P, CC, P], bt16, tag="w1m", name="w1m")
            nc.sync.dma_start(nw[:], g.w1[:, mm + 3])
            w1tiles.append(nw)
        if mm == MM - 2:
            # prefetch the first W2 chunks behind the last W1 loads
            w2tiles = []
            for oc in range(2):
                w2m = g.w2s.tile([P, MM, P], bt16, tag="w2m", name="w2m")
                nc.sync.dma_start(w2m[:], g.w2[:, oc])
                w2tiles.append(w2m)
        p1 = g.ps.tile([P, 512], f32, tag="pp", name="ps_f1")
        for c in range(CC):
            nc.tensor.matmul(p1[:], w1m[:, c, :], g.x3h[:, c, :],
                             start=(c == 0), stop=(c == CC - 1))
        nc.scalar.activation(out=h1[mm][:], in_=p1[:], func=AF.Gelu,
                             bias=g.vec["b1"][:, mm:mm + 1], scale=1.0)
    for oc in range(CC):
        w2m = w2tiles[oc]
        if oc + 2 < CC:
            nw = g.w2s.tile([P, MM, P], bt16, tag="w2m", name="w2m")
            nc.sync.dma_start(nw[:], g.w2[:, oc + 2])
            w2tiles.append(nw)
        p2 = g.ps.tile([P, 512], f32, tag="pp", name="ps_f2")
        for k in range(MM):
            nc.tensor.matmul(p2[:], w2m[:, k, :], h1[k][:, :],
                             start=(k == 0), stop=(k == MM - 1))
        # out = gamma2 * xhat2 + (ffw + b2 + beta2)
        t = g.x2p.tile([P, 512], f32, tag="x2t", name="x2o")
        nc.vector.scalar_tensor_tensor(
            out=t[:], in0=g.x3h[:, oc, :], scalar=g.vec["g2"][:, oc:oc + 1],
            in1=p2[:], op0=OP.mult, op1=OP.add)
        ot = g.outp.tile([P, 512], f32, tag="otile", name="otile")
        nc.vector.tensor_scalar(
            out=ot[:], in0=t[:], scalar1=g.vec["b2s"][:, oc:oc + 1],
            scalar2=None, op0=OP.add)
        nc.sync.dma_start(g.out[oc], ot[:])


def build_kernel():
    nc = bass.Bass("TRN2", target_bir_lowering=False, num_devices=8)
    g = Ctx()
    g.nc = nc

    g.xc = nc.dram_tensor("xc", [T, C], f32, kind="ExternalInput").ap()
    g.wk = nc.dram_tensor("wk", [P, CC, CC, P], bt16, kind="ExternalInput").ap()
    g.wq = nc.dram_tensor("wq", [P, CC, CC, P], bt16, kind="ExternalInput").ap()
    g.wv = nc.dram_tensor("wv", [P, CC, C], bt16, kind="ExternalInput").ap()
    g.wo = nc.dram_tensor("wo", [P, CC, CC, P], bt16, kind="ExternalInput").ap()
    g.w1 = nc.dram_tensor("w1", [P, MM, CC, P], bt16, kind="ExternalInput").ap()
    g.w2 = nc.dram_tensor("w2", [P, CC, MM, P], bt16, kind="ExternalInput").ap()
    g.masks = nc.dram_tensor("masks", [P, 16, QB], bt16,
                             kind="ExternalInput").ap()
    vecs = {}
    for nm, n in [("bq", CC), ("bk", CC), ("bos", CC), ("b1", MM),
                  ("b2s", CC), ("g1", CC), ("g2", CC)]:
        vecs[nm] = nc.dram_tensor(nm, [n, P], f32, kind="ExternalInput").ap()
    g.out = nc.dram_tensor("out", [CC, P, R], f32, kind="ExternalOutput").ap()

    with tile.TileContext(nc) as tc:
        g.tc = tc
        _build_body(g, vecs)
    _split_sync_waits(nc)
    return nc


def _build_body(g, vecs):
    nc, tc = g.nc, g.tc
    from contextlib import ExitStack
    with ExitStack() as es:
        pool = lambda name, bufs, **kw: es.enter_context(
            tc.tile_pool(name=name, bufs=bufs, **kw))
        setup = pool("setup", 1)
        g.setup = setup
        g.stats = pool("stats", 4)
        g.ps = pool("ps", 2, space="PSUM")
        g.xnp = pool("xnp", 2)
        g.rbcp = pool("rbcp", 2)
        hp = pool("hp", 1)
        g.x2p = pool("x2p", 2)
        x2Tp = pool("x2Tp", 1)
        x3p = pool("x3p", 1)

        ident = setup.tile([P, P], bt16, tag="ident", name="ident")
        make_identity(nc, ident[:])
        g.ident = ident
        g.eps_sb = setup.tile([P, 1], f32, tag="eps", name="eps")
        nc.vector.memset(g.eps_sb[:], EPS)
        g.ones64 = setup.tile([1, DH], bt16, tag="ones64", name="ones64")
        nc.vector.memset(g.ones64[:], 1.0)
        g.ones128 = setup.tile([P, 1], bt16, tag="ones128", name="ones128")
        nc.vector.memset(g.ones128[:], 1.0)
        g.onesPf = setup.tile([1, P], f32, tag="onesPf", name="onesPf")
        nc.vector.memset(g.onesPf[:], 1.0)
        # warm the PE p-state while the first x DMA is in flight
        pwarm = g.ps.tile([P, 2048], bt16, tag="psc", name="pwarm")
        for i in range(16):
            nc.tensor.transpose(pwarm[:, i * P:(i + 1) * P], ident[:],
                                ident[:])

        g.hcat = [hp.tile([P, 512], bt16, tag=f"hcat{c}", name=f"hcat{c}")
                  for c in range(CC)]
        g.x2T = [x2Tp.tile([P, 512], bt16, tag=f"x2T{mo}", name=f"x2T{mo}")
                 for mo in range(CC)]
        g.x3h = x3p.tile([P, CC, 512], bt16, tag="x3h", name="x3h")

        with ExitStack() as es2:
            pool2 = lambda name, bufs: es2.enter_context(
                tc.tile_pool(name=name, bufs=bufs))
            x1p = pool2("x1p", 1)
            kvp = pool2("kvp", 1)
            g.wstr = pool2("wstr", 2)
            g.x1T = [x1p.tile([P, CC, 512], bt16, tag=f"x1T{rb}",
                              name=f"x1T{rb}") for rb in range(4)]
            g.vv = [kvp.tile([P, H, DH + 1], bt16, tag=f"vv{kt}",
                             name=f"vv{kt}") for kt in range(T // P)]
            with ExitStack() as esa:
                poola = lambda name, bufs: esa.enter_context(
                    tc.tile_pool(name=name, bufs=bufs))
                g.xio = poola("xio", 3)
                wvp = poola("wvp", 1)
                # first xin DMAs go out before anything else
                g.xt_pre = []
                for rt in range(3):
                    xt = g.xio.tile([P, C], f32, tag="xin", name="xin")
                    _xin_dma(g, xt, rt)
                    g.xt_pre.append(xt)
                g.wvs = wvp.tile([P, CC, C], bt16, tag="wvs", name="wvs")
                nc.sync.dma_start(g.wvs[:], g.wv)
                g.vec = {}
                for nm, ap_ in vecs.items():
                    n = ap_.shape[0]
                    t = g.setup.tile([P, n], f32, tag=f"vec_{nm}",
                                     name=f"vec_{nm}")
                    nc.sync.dma_start(t[:], ap_.rearrange("c p -> p c"))
                    g.vec[nm] = t
                _phase_a(g)
            with ExitStack() as esb:
                poolb = lambda name, bufs: esb.enter_context(
                    tc.tile_pool(name=name, bufs=bufs))
                g.kvq = poolb("kvq", 3)
                mp = poolb("mp", 1)
                g.arnd = poolb("arp", 6)
                g.mq = mp.tile([P, 16, QB], bt16, tag="mask", name="mask")
                nc.sync.dma_start(g.mq[:], g.masks)
                _phase_b(g)
                _phase_c(g)
        _phase_d(g)


def _phase_b(g):
    kq = {}
    pending = []          # [(pair_index, chain_closure), ...] in order

    def ensure(mi):
        if mi < CC and mi not in kq:
            kT, qT, chains = _kq_chains(g, mi)
            kq[mi] = (kT, qT)
            pending.extend((mi, ch) for ch in chains)

    def filler():
        if pending:
            pending.pop(0)[1]()

    norms = []

    def deferred_norm():
        if norms:
            norms.pop(0)()

    ensure(0)
    while pending:
        pending.pop(0)[1]()
    ensure(1)
    for m in range(CC):
        ensure(m + 2)
        kT, qT = kq[m]
        for hl in range(2):
            norms.append(_attn_head(g, kT, qT, m, hl, filler, deferred_norm))
        # everything pair m+1 references must be issued before its scores
        while pending and pending[0][0] <= m + 1:
            pending.pop(0)[1]()
    while norms:
        norms.pop(0)()


_NC_CACHE = None


def _get_nc():
    global _NC_CACHE
    if _NC_CACHE is None:
        _NC_CACHE = build_kernel()
    return _NC_CACHE


def _prep_shared(inputs):
    scale = DH ** -0.5
    Wq = np.asarray(inputs["Wq"], np.float32)
    Wk = np.asarray(inputs["Wk"], np.float32)
    Wv = np.asarray(inputs["Wv"], np.float32)
    Wo = np.asarray(inputs["Wo"], np.float32)
    W1 = np.asarray(inputs["W1"], np.float32)
    W2 = np.asarray(inputs["W2"], np.float32)
    g1 = np.asarray(inputs["gamma1"], np.float32)
    be1 = np.asarray(inputs["beta1"], np.float32)
    g2 = np.asarray(inputs["gamma2"], np.float32)
    be2 = np.asarray(inputs["beta2"], np.float32)

    Wk2 = np.ascontiguousarray(Wk.transpose(1, 0, 2).reshape(C, C))
    Wq2 = np.ascontiguousarray(Wq.transpose(1, 0, 2).reshape(C, C)) * scale
    Wv2 = np.ascontiguousarray(Wv.transpose(1, 0, 2).reshape(C, C))

    bk_eff = np.asarray(inputs["bk"], np.float32).reshape(C) + be1 @ Wk2
    bq_eff = (np.asarray(inputs["bq"], np.float32).reshape(C) * scale
              + be1 @ Wq2)
    bv_eff = np.asarray(inputs["bv"], np.float32).reshape(C) + be1 @ Wv2
    bo_eff = np.asarray(inputs["bo"], np.float32) + bv_eff @ Wo
    bos = bo_eff + be1                      # residual: + beta1 + bo
    b1_eff = np.asarray(inputs["b1"], np.float32) + be2 @ W1
    b2s = np.asarray(inputs["b2"], np.float32) + be2   # + beta2 + b2

    wk_eff = Wk2 * g1[:, None]
    wq_eff = Wq2 * g1[:, None]
    wv_eff = Wv2 * g1[:, None]
    w1_eff = W1 * g2[:, None]

    def chunk4(W, n_out):  # [C, n_out*128] -> [128p, n_out, C//128, 128]
        return np.ascontiguousarray(
            W.reshape(W.shape[0] // P, P, n_out, P).transpose(1, 2, 0, 3)
        ).astype(bf16)

    shared = {
        "wk": chunk4(wk_eff, CC),
        "wq": chunk4(wq_eff, CC),
        "wv": np.ascontiguousarray(
            wv_eff.reshape(CC, P, C).transpose(1, 0, 2)).astype(bf16),
        "wo": chunk4(Wo, CC),
        "w1": chunk4(w1_eff, MM),
        "w2": chunk4(W2, CC),
        "bq": bq_eff.reshape(CC, P).copy(),
        "bk": bk_eff.reshape(CC, P).copy(),
        "bos": bos.reshape(CC, P).copy(),
        "b1": b1_eff.reshape(MM, P).copy(),
        "b2s": b2s.reshape(CC, P).copy(),
        "g1": g1.reshape(CC, P).copy(),
        "g2": g2.reshape(CC, P).copy(),
    }
    return shared


def _core_order(j):
    return [j, 7 - j] + sorted(set(range(8)) - {j, 7 - j})


def _core_masks(j):
    """[128, 16, 256] bf16. Slots 0..7: early-half (query block j) masks for
    the BOTH chunks; slots 8..15: late-half (query block 7-j) masks for the
    LATE chunks. Chunk kc holds permuted keys order[kc//2]*256+(kc%2)*128+p."""
    order = _core_order(j)
    out = np.zeros((P, 16, QB), np.float32)
    for s, kc in enumerate(BOTH):
        key = order[kc // 2] * QB + (kc % 2) * P + np.arange(P)[:, None]
        qglob = j * QB + np.arange(QB)[None, :]
        out[:, s, :] = (key <= qglob)
    for s, kc in enumerate(LATE):
        key = order[kc // 2] * QB + (kc % 2) * P + np.arange(P)[:, None]
        qglob = (7 - j) * QB + np.arange(QB)[None, :]
        out[:, 8 + s, :] = (key <= qglob)
    return out.astype(bf16)


def _make_in_maps(inputs):
    x = np.asarray(inputs["x"], np.float32)
    shared = _prep_shared(inputs)
    in_maps = []
    for c in range(8):
        gg, j = c // 4, c % 4
        xb = x[gg]
        order = _core_order(j)
        m = dict(shared)
        m["xc"] = np.ascontiguousarray(
            np.concatenate([xb[b * QB:(b + 1) * QB] for b in order], 0))
        m["masks"] = _core_masks(j)
        in_maps.append(m)
    return in_maps


def _assemble(results):
    out = np.zeros((B, T, C), np.float32)
    for c in range(8):
        gg, j = c // 4, c % 4
        o = results[c]["out"].reshape(C, R).T  # [512, C] rows = 2 blocks
        out[gg, j * QB:(j + 1) * QB] = o[:QB]
        out[gg, (7 - j) * QB:(8 - j) * QB] = o[QB:]
    return out


def kernel(**inputs):
    in_maps = _make_in_maps(inputs)
    nc = _get_nc()
    res = run_bass_kernel_spmd(nc, in_maps, core_ids=list(range(8)))
    return _assemble(res.results)
